# revision 1
# baseline (speedup 1.0000x reference)
"""Graphormer layer (pre-norm MHSA + additive attn bias + SiLU FFN) on 8 trn2 cores.

Sharding: core c handles batch b = c//4 and query rows i0 = (c%4)*512 .. +512.
Each core computes LN1 + full K/V for its batch (replicated inside the
4-core batch group), Q/scores/softmax/attn@V for its 512 query rows, the
output projection, LN2 and the full FFN for those rows.  No collectives.

Host-side prep rotates each core's token axis by -i0 so the query block is
always columns 0:512 of the same SPMD program; the attn-bias j axis is
rotated identically (softmax/attn@V are order-invariant over j).

Layouts on device are feature-major ("transposed"): xT [D, T], qT/kT [d, T],
scoresT [j, i].  The softmax denominator comes from appending a ones column
to V in the attn@V matmul; normalization uses a gpsimd partition-broadcast
of the reciprocal.  Matmul operands are bf16 (fp32 accumulation in PSUM);
the residual path stays fp32.  Softmax skips the max-subtraction: scores
are O(8) here so exp stays comfortably inside fp32 range.
"""

import sys
from contextlib import ExitStack

import numpy as np

sys.path.insert(0, "/opt/trn_rl_repo")

import ml_dtypes  # noqa: E402

import concourse.bass as bass  # noqa: E402
import concourse.bacc as bacc  # noqa: E402
import concourse.tile as tile  # noqa: E402
from concourse import mybir  # noqa: E402
from concourse.bass_utils import run_bass_kernel_spmd  # noqa: E402

F32 = mybir.dt.float32
BF16 = mybir.dt.bfloat16
AF = mybir.ActivationFunctionType
OP = mybir.AluOpType
BF16_NP = ml_dtypes.bfloat16

B, T, D = 2, 2048, 1024
H, HD = 16, 64
FF = 4 * D
N_CORES = 8
IB = 512           # query rows per core
SCALE = 1.0 / 8.0  # 1/sqrt(HD)
EPS = 1e-5

_cache = {}


def build_program():
    nc = bacc.Bacc("TRN2", target_bir_lowering=False, debug=False)

    # ---- DRAM I/O ----
    xT_d = nc.dram_tensor("xT", [D, T], F32, kind="ExternalInput").ap()
    biasT_d = nc.dram_tensor("biasT", [H, T, IB], BF16, kind="ExternalInput").ap()
    Wq_d = nc.dram_tensor("Wq", [D, D], BF16, kind="ExternalInput").ap()
    Wk_d = nc.dram_tensor("Wk", [D, D], BF16, kind="ExternalInput").ap()
    Wv_d = nc.dram_tensor("Wv", [D, D], BF16, kind="ExternalInput").ap()
    Wo_d = nc.dram_tensor("Wo", [D, D], BF16, kind="ExternalInput").ap()
    W1_d = nc.dram_tensor("W1", [D, FF], BF16, kind="ExternalInput").ap()
    W2_d = nc.dram_tensor("W2", [FF, D], BF16, kind="ExternalInput").ap()
    # packed per-partition params: [128, n_tiles] fp32
    g1_d = nc.dram_tensor("g1", [128, 8], F32, kind="ExternalInput").ap()
    bg1_d = nc.dram_tensor("bg1", [128, 8], F32, kind="ExternalInput").ap()
    g2_d = nc.dram_tensor("g2", [128, 8], F32, kind="ExternalInput").ap()
    bg2_d = nc.dram_tensor("bg2", [128, 8], F32, kind="ExternalInput").ap()
    bq8_d = nc.dram_tensor("bq8", [128, 8], F32, kind="ExternalInput").ap()
    bk_d = nc.dram_tensor("bk", [128, 8], F32, kind="ExternalInput").ap()
    bo_d = nc.dram_tensor("bo", [128, 8], F32, kind="ExternalInput").ap()
    b1_d = nc.dram_tensor("b1", [128, 32], F32, kind="ExternalInput").ap()
    b2_d = nc.dram_tensor("b2", [128, 8], F32, kind="ExternalInput").ap()
    bv_d = nc.dram_tensor("bv", [1, D], BF16, kind="ExternalInput").ap()
    outT_d = nc.dram_tensor("outT", [D, IB], F32, kind="ExternalOutput").ap()

    with tile.TileContext(nc) as tc, ExitStack() as ctx:
        # ---------------- outermost (whole-kernel lifetime) ----------------
        const_p = ctx.enter_context(tc.tile_pool(name="const", bufs=1))
        param_p = ctx.enter_context(tc.tile_pool(name="param", bufs=1))
        res_p = ctx.enter_context(tc.tile_pool(name="res", bufs=1))
        oT_p = ctx.enter_context(tc.tile_pool(name="oT", bufs=1))
        out_p = ctx.enter_context(tc.tile_pool(name="out", bufs=2))

        ones_f = const_p.tile([128, 128], F32, tag="ones_f")
        nc.vector.memset(ones_f[:], 1.0)
        ones_b = const_p.tile([1, 128], BF16, tag="ones_b")
        nc.vector.memset(ones_b[:], 1.0)
        eps_t = const_p.tile([1, 1], F32, tag="eps")
        nc.vector.memset(eps_t[:], EPS)

        def load_param(name, dram, shape, dtype=F32):
            t = param_p.tile(shape, dtype, tag=name, name=name)
            nc.sync.dma_start(t[:], dram[:])
            return t

        g1 = load_param("g1", g1_d, [128, 8])
        bg1 = load_param("bg1", bg1_d, [128, 8])
        g2 = load_param("g2", g2_d, [128, 8])
        bg2 = load_param("bg2", bg2_d, [128, 8])
        bq8 = load_param("bq8", bq8_d, [128, 8])
        bk = load_param("bk", bk_d, [128, 8])
        bo = load_param("bo", bo_d, [128, 8])
        b1 = load_param("b1", b1_d, [128, 32])
        b2 = load_param("b2", b2_d, [128, 8])
        bv = load_param("bv", bv_d, [1, D], BF16)

        # res: x residual slice in phases A-D, then reused in place as
        # xres = x + attn_out for phases D-E.
        res = [res_p.tile([128, IB], F32, tag=f"res{e}", name=f"res{e}")
               for e in range(8)]
        oT = [oT_p.tile([128, IB], BF16, tag=f"oT{d}", name=f"oT{d}")
              for d in range(8)]

        # ---------------- scope: K/V/Q (phases A-C) ------------------------
        with tc.tile_pool(name="kT", bufs=1) as kT_p, \
             tc.tile_pool(name="vcat", bufs=1) as vcat_p, \
             tc.tile_pool(name="qT", bufs=1) as qT_p:
            kT = [kT_p.tile([128, T], BF16, tag=f"kT{d}", name=f"kT{d}")
                  for d in range(8)]
            vcat = [vcat_p.tile([128, H * (HD + 1)], BF16, tag=f"vc{t}",
                                name=f"vc{t}") for t in range(16)]
            qT = [qT_p.tile([128, IB], BF16, tag=f"qT{d}", name=f"qT{d}")
                  for d in range(8)]

            # ---------------- scope: hT (phases A-B) -----------------------
            with tc.tile_pool(name="hT", bufs=1) as hT_p:
                hT = [hT_p.tile([128, T], BF16, tag=f"hT{e}", name=f"hT{e}")
                      for e in range(8)]

                # ===== Phase A: LN1 (streamed, partition-axis stats) =======
                with tc.tile_pool(name="xc", bufs=2) as xc_p, \
                     tc.tile_pool(name="sq", bufs=3) as sq_p, \
                     tc.tile_pool(name="lnt", bufs=2) as lnt_p, \
                     tc.tile_pool(name="lnb", bufs=2) as lnb_p, \
                     tc.tile_pool(name="lnps", bufs=2,
                                  space=bass.MemorySpace.PSUM) as lnps_p:
                    for n in range(4):
                        nb = slice(n * 512, (n + 1) * 512)
                        xcs = []
                        ps_mu = lnps_p.tile([1, 512], F32, tag="psmu", name="psmu")
                        ps_sq = lnps_p.tile([1, 512], F32, tag="pssq", name="pssq")
                        for e in range(8):
                            xc = xc_p.tile([128, 512], F32, tag=f"xc{e}", name="xc")
                            nc.sync.dma_start(xc[:], xT_d[e * 128:(e + 1) * 128, nb])
                            xcs.append(xc)
                            nc.tensor.matmul(ps_mu[:], ones_f[:, 0:1], xc[:],
                                             start=(e == 0), stop=(e == 7))
                            x2 = sq_p.tile([128, 512], F32, tag="x2", name="x2")
                            nc.scalar.square(x2[:], xc[:])
                            nc.tensor.matmul(ps_sq[:], ones_f[:, 0:1], x2[:],
                                             start=(e == 0), stop=(e == 7))
                        mu_n = lnt_p.tile([1, 512], F32, tag="mu_n", name="mu_n")
                        nc.scalar.activation(mu_n[:], ps_mu[:], AF.Identity,
                                             scale=1.0 / D)
                        mu2_n = lnt_p.tile([1, 512], F32, tag="mu2_n", name="mu2_n")
                        nc.scalar.square(mu2_n[:], mu_n[:])
                        var_n = lnt_p.tile([1, 512], F32, tag="var_n", name="var_n")
                        nc.vector.scalar_tensor_tensor(
                            var_n[:], ps_sq[:], 1.0 / D, mu2_n[:],
                            op0=OP.mult, op1=OP.subtract)
                        std_n = lnt_p.tile([1, 512], F32, tag="std_n", name="std_n")
                        nc.scalar.activation(std_n[:], var_n[:], AF.Sqrt, bias=eps_t[:])
                        rstd_n = lnt_p.tile([1, 512], F32, tag="rstd_n", name="rstd_n")
                        nc.vector.reciprocal(rstd_n[:], std_n[:])
                        mu_b = lnb_p.tile([128, 512], F32, tag="mu_b", name="mu_b")
                        nc.gpsimd.partition_broadcast(mu_b[:], mu_n[:])
                        rstd_b = lnb_p.tile([128, 512], F32, tag="rstd_b",
                                            name="rstd_b")
                        nc.gpsimd.partition_broadcast(rstd_b[:], rstd_n[:])
                        for e in range(8):
                            if n == 0:
                                nc.scalar.activation(res[e][:], xcs[e][:],
                                                     AF.Identity)
                            t = sq_p.tile([128, 512], F32, tag="lnap", name="lnap")
                            nc.vector.tensor_sub(t[:], xcs[e][:], mu_b[:])
                            nc.vector.tensor_mul(t[:], t[:], rstd_b[:])
                            nc.scalar.activation(hT[e][:, nb], t[:], AF.Identity,
                                                 scale=g1[:, e:e + 1],
                                                 bias=bg1[:, e:e + 1])

                # ===== Phase B: Q/K/V projections ==========================
                with tc.tile_pool(name="wp", bufs=12) as wp, \
                     tc.tile_pool(name="wv512", bufs=2) as wv_p, \
                     tc.tile_pool(name="pps", bufs=4,
                                  space=bass.MemorySpace.PSUM) as pps:
                    # qT[d, i] for this core's rows (= token cols 0:IB)
                    for dt in range(8):
                        ps = pps.tile([128, 512], F32, tag="ps", name="psq")
                        for e in range(8):
                            wt = wp.tile([128, 128], BF16, tag="w", name="wq")
                            nc.sync.dma_start(
                                wt[:], Wq_d[e * 128:(e + 1) * 128,
                                            dt * 128:(dt + 1) * 128])
                            nc.tensor.matmul(ps[:], wt[:], hT[e][:, 0:IB],
                                             start=(e == 0), stop=(e == 7))
                        nc.scalar.activation(qT[dt][:], ps[:], AF.Identity,
                                             scale=SCALE, bias=bq8[:, dt:dt + 1])
                    # kT[d, j] over all tokens
                    for dt in range(8):
                        for n in range(4):
                            nb = slice(n * 512, (n + 1) * 512)
                            ps = pps.tile([128, 512], F32, tag="ps", name="psk")
                            for e in range(8):
                                wt = wp.tile([128, 128], BF16, tag="w", name="wk")
                                nc.sync.dma_start(
                                    wt[:], Wk_d[e * 128:(e + 1) * 128,
                                                dt * 128:(dt + 1) * 128])
                                nc.tensor.matmul(ps[:], wt[:], hT[e][:, nb],
                                                 start=(e == 0), stop=(e == 7))
                            nc.scalar.activation(kT[dt][:, nb], ps[:], AF.Identity,
                                                 bias=bk[:, dt:dt + 1])
                    # v[j, d] natural layout + ones column per head
                    for tt in range(16):
                        nc.vector.memset(
                            vcat[tt][:].rearrange(
                                "p (h x) -> p h x", x=HD + 1)[:, :, HD:HD + 1],
                            1.0)
                    for n in range(2):
                        nb = slice(n * 512, (n + 1) * 512)
                        wv_tiles = []
                        for e in range(8):
                            wv = wv_p.tile([128, 512], BF16, tag=f"wv{e}",
                                           name=f"wv{e}")
                            nc.sync.dma_start(wv[:], Wv_d[e * 128:(e + 1) * 128, nb])
                            wv_tiles.append(wv)
                        for tt in range(16):
                            tb = slice(tt * 128, (tt + 1) * 128)
                            ps = pps.tile([128, 512], F32, tag="ps", name="psv")
                            for e in range(8):
                                nc.tensor.matmul(ps[:], hT[e][:, tb],
                                                 wv_tiles[e][:],
                                                 start=(e == 0), stop=False)
                            nc.tensor.matmul(ps[:], ones_b[:], bv[:, nb],
                                             start=False, stop=True)
                            dst = vcat[tt][:, n * 8 * (HD + 1):(n + 1) * 8 * (HD + 1)]
                            dst = dst.rearrange("p (h x) -> p h x",
                                                x=HD + 1)[:, :, 0:HD]
                            src = ps[:].rearrange("p (h d) -> p h d", d=HD)
                            nc.scalar.activation(dst, src, AF.Identity)
            # hT pool closed here

            # ===== Phase C: attention ======================================
            with tc.tile_pool(name="biasdma", bufs=8) as bias_p, \
                 tc.tile_pool(name="upre", bufs=4) as up_p, \
                 tc.tile_pool(name="uexp", bufs=4) as u_p, \
                 tc.tile_pool(name="nrm", bufs=2) as nrm_p, \
                 tc.tile_pool(name="pss", bufs=2,
                              space=bass.MemorySpace.PSUM) as pss, \
                 tc.tile_pool(name="pso", bufs=2,
                              space=bass.MemorySpace.PSUM) as pso:
                for h in range(H):
                    dt, po = h // 2, (h % 2) * 64
                    ps_o = pso.tile([HD + 1, 512], F32, tag="ps_o", name="ps_o")
                    for j in range(16):
                        jb = slice(j * 128, (j + 1) * 128)
                        ps_s = pss.tile([128, 512], F32, tag="ps_s", name="ps_s")
                        nc.tensor.matmul(ps_s[:], kT[dt][po:po + 64, jb],
                                         qT[dt][po:po + 64, :],
                                         start=True, stop=True)
                        bt = bias_p.tile([128, IB], BF16, tag="bt", name="bt")
                        nc.sync.dma_start(bt[:], biasT_d[h, jb, :])
                        up = up_p.tile([128, IB], F32, tag="up", name="up")
                        nc.vector.scalar_tensor_tensor(up[:], ps_s[:], 1.0, bt[:],
                                                       op0=OP.mult, op1=OP.add)
                        u = u_p.tile([128, IB], BF16, tag="u", name="u")
                        nc.scalar.activation(u[:], up[:], AF.Exp)
                        nc.tensor.matmul(
                            ps_o[:], vcat[j][:, h * (HD + 1):(h + 1) * (HD + 1)],
                            u[:], start=(j == 0), stop=(j == 15))
                    recip = nrm_p.tile([1, 512], F32, tag="recip", name="recip")
                    nc.vector.reciprocal(recip[:], ps_o[64:65, :])
                    rb = nrm_p.tile([64, 512], F32, tag="rb", name="rb")
                    nc.gpsimd.partition_broadcast(rb[:], recip[:])
                    nc.vector.tensor_mul(oT[dt][po:po + 64, :], ps_o[0:64, :],
                                         rb[:])
        # kT/vcat/qT pools closed here

        # ---------------- scope: h2/sz (phases D-E) ------------------------
        with tc.tile_pool(name="h2", bufs=1) as h2_p, \
             tc.tile_pool(name="sz", bufs=1) as sz_p:
            h2 = [h2_p.tile([128, IB], BF16, tag=f"h2{e}", name=f"h2{e}")
                  for e in range(8)]
            sz = [sz_p.tile([128, IB], BF16, tag=f"sz{f}", name=f"sz{f}")
                  for f in range(32)]

            # ===== Phase D: out-projection + LN2 ===========================
            with tc.tile_pool(name="wp2", bufs=12) as wp2, \
                 tc.tile_pool(name="sq2", bufs=3) as sq2_p, \
                 tc.tile_pool(name="lnt2", bufs=2) as lnt2_p, \
                 tc.tile_pool(name="lnb2", bufs=2) as lnb2_p, \
                 tc.tile_pool(name="dps", bufs=2,
                              space=bass.MemorySpace.PSUM) as dps, \
                 tc.tile_pool(name="dps1", bufs=1,
                              space=bass.MemorySpace.PSUM) as dps1:
                for et in range(8):
                    ps = dps.tile([128, 512], F32, tag="psx1", name="psx1")
                    for dt in range(8):
                        wt = wp2.tile([128, 128], BF16, tag="w2", name="wo")
                        nc.sync.dma_start(wt[:], Wo_d[dt * 128:(dt + 1) * 128,
                                                      et * 128:(et + 1) * 128])
                        nc.tensor.matmul(ps[:], wt[:], oT[dt][:],
                                         start=(dt == 0), stop=(dt == 7))
                    # res[et] <- x + attn_out (+bo), in place
                    nc.vector.scalar_tensor_tensor(res[et][:], ps[:],
                                                   bo[:, et:et + 1], res[et][:],
                                                   op0=OP.add, op1=OP.add)
                # LN2 (single 512-col block)
                ps_mu = dps1.tile([1, 512], F32, tag="psmu2", name="psmu2")
                for e in range(8):
                    nc.tensor.matmul(ps_mu[:], ones_f[:, 0:1], res[e][:],
                                     start=(e == 0), stop=(e == 7))
                ps_sq = dps1.tile([1, 512], F32, tag="pssq2", name="pssq2")
                for e in range(8):
                    x2 = sq2_p.tile([128, 512], F32, tag="x22", name="x22")
                    nc.scalar.square(x2[:], res[e][:])
                    nc.tensor.matmul(ps_sq[:], ones_f[:, 0:1], x2[:],
                                     start=(e == 0), stop=(e == 7))
                mu_n = lnt2_p.tile([1, 512], F32, tag="mu_n2", name="mu_n2")
                nc.scalar.activation(mu_n[:], ps_mu[:], AF.Identity, scale=1.0 / D)
                mu2_n = lnt2_p.tile([1, 512], F32, tag="mu2_n2", name="mu2_n2")
                nc.scalar.square(mu2_n[:], mu_n[:])
                var_n = lnt2_p.tile([1, 512], F32, tag="var_n2", name="var_n2")
                nc.vector.scalar_tensor_tensor(var_n[:], ps_sq[:], 1.0 / D,
                                               mu2_n[:], op0=OP.mult,
                                               op1=OP.subtract)
                std_n = lnt2_p.tile([1, 512], F32, tag="std_n2", name="std_n2")
                nc.scalar.activation(std_n[:], var_n[:], AF.Sqrt, bias=eps_t[:])
                rstd_n = lnt2_p.tile([1, 512], F32, tag="rstd_n2", name="rstd_n2")
                nc.vector.reciprocal(rstd_n[:], std_n[:])
                mu_b = lnb2_p.tile([128, 512], F32, tag="mu_b2", name="mu_b2")
                nc.gpsimd.partition_broadcast(mu_b[:], mu_n[:])
                rstd_b = lnb2_p.tile([128, 512], F32, tag="rstd_b2",
                                     name="rstd_b2")
                nc.gpsimd.partition_broadcast(rstd_b[:], rstd_n[:])
                for e in range(8):
                    t = sq2_p.tile([128, IB], F32, tag="lnap2", name="lnap2")
                    nc.vector.tensor_sub(t[:], res[e][:], mu_b[:])
                    nc.vector.tensor_mul(t[:], t[:], rstd_b[:])
                    nc.scalar.activation(h2[e][:], t[:], AF.Identity,
                                         scale=g2[:, e:e + 1],
                                         bias=bg2[:, e:e + 1])

            # ===== Phase E: FFN ============================================
            with tc.tile_pool(name="wp3", bufs=16) as wp3, \
                 tc.tile_pool(name="sg", bufs=3) as sg_p, \
                 tc.tile_pool(name="eps", bufs=4,
                              space=bass.MemorySpace.PSUM) as eps_p:
                for ft in range(32):
                    ps = eps_p.tile([128, 512], F32, tag="pse", name="psz")
                    for e in range(8):
                        wt = wp3.tile([128, 128], BF16, tag="w3", name="w1t")
                        nc.sync.dma_start(wt[:], W1_d[e * 128:(e + 1) * 128,
                                                      ft * 128:(ft + 1) * 128])
                        nc.tensor.matmul(ps[:], wt[:], h2[e][:],
                                         start=(e == 0), stop=(e == 7))
                    sg = sg_p.tile([128, IB], BF16, tag="sg", name="sg")
                    nc.scalar.activation(sg[:], ps[:], AF.Sigmoid,
                                         bias=b1[:, ft:ft + 1])
                    # silu(z) = z * sigmoid(z), z = ps + b1
                    nc.vector.scalar_tensor_tensor(sz[ft][:], ps[:],
                                                   b1[:, ft:ft + 1], sg[:],
                                                   op0=OP.add, op1=OP.mult)
                for et in range(8):
                    ps = eps_p.tile([128, 512], F32, tag="pse", name="psy")
                    for ft in range(32):
                        wt = wp3.tile([128, 128], BF16, tag="w3", name="w2t")
                        nc.sync.dma_start(wt[:], W2_d[ft * 128:(ft + 1) * 128,
                                                      et * 128:(et + 1) * 128])
                        nc.tensor.matmul(ps[:], wt[:], sz[ft][:],
                                         start=(ft == 0), stop=(ft == 31))
                    ot = out_p.tile([128, IB], F32, tag="outt", name="outt")
                    nc.vector.scalar_tensor_tensor(ot[:], ps[:], b2[:, et:et + 1],
                                                   res[et][:], op0=OP.add,
                                                   op1=OP.add)
                    nc.sync.dma_start(outT_d[et * 128:(et + 1) * 128, :], ot[:])

    nc.compile()
    return nc


def _prep_inputs(inputs):
    """Host-side layout prep -> list of 8 per-core input maps."""
    x = np.asarray(inputs["x"], dtype=np.float32)
    ab = np.asarray(inputs["attn_bias"], dtype=np.float32)

    def pack(v, ntiles):
        return np.ascontiguousarray(
            np.asarray(v, np.float32).reshape(ntiles, 128).T)

    shared = {
        "Wq": np.ascontiguousarray(np.asarray(inputs["Wq"]).astype(BF16_NP)),
        "Wk": np.ascontiguousarray(np.asarray(inputs["Wk"]).astype(BF16_NP)),
        "Wv": np.ascontiguousarray(np.asarray(inputs["Wv"]).astype(BF16_NP)),
        "Wo": np.ascontiguousarray(np.asarray(inputs["Wo"]).astype(BF16_NP)),
        "W1": np.ascontiguousarray(np.asarray(inputs["W1"]).astype(BF16_NP)),
        "W2": np.ascontiguousarray(np.asarray(inputs["W2"]).astype(BF16_NP)),
        "g1": pack(inputs["ln1_g"], 8),
        "bg1": pack(inputs["ln1_b"], 8),
        "g2": pack(inputs["ln2_g"], 8),
        "bg2": pack(inputs["ln2_b"], 8),
        "bq8": pack(np.asarray(inputs["bq"], np.float32) * SCALE, 8),
        "bk": pack(inputs["bk"], 8),
        "bo": pack(inputs["bo"], 8),
        "b1": pack(inputs["b1"], 32),
        "b2": pack(inputs["b2"], 8),
        "bv": np.ascontiguousarray(
            np.asarray(inputs["bv"], np.float32).astype(BF16_NP).reshape(1, D)),
    }
    xT = [np.ascontiguousarray(x[b].T) for b in range(B)]  # [D, T] f32
    ab_bf = ab.astype(BF16_NP)  # [B, H, T(i), T(j)]
    in_maps = []
    for c in range(N_CORES):
        b, i0 = c // 4, (c % 4) * IB
        # token axis rotated by -i0 (queries land at cols 0:IB); the j axis
        # of the bias is rotated identically to match k/v token order.
        xTc = np.ascontiguousarray(np.roll(xT[b], -i0, axis=1))
        biasT = np.ascontiguousarray(
            np.roll(ab_bf[b, :, i0:i0 + IB, :], -i0, axis=2).transpose(0, 2, 1))
        m = {"xT": xTc, "biasT": biasT}
        m.update(shared)
        in_maps.append(m)
    return in_maps


def kernel(**inputs):
    if "nc" not in _cache:
        _cache["nc"] = build_program()
    nc = _cache["nc"]
    in_maps = _prep_inputs(inputs)
    r = run_bass_kernel_spmd(nc, in_maps, list(range(N_CORES)))
    out = np.empty((B, T, D), dtype=np.float32)
    for c in range(N_CORES):
        b, i0 = c // 4, (c % 4) * IB
        out[b, i0:i0 + IB, :] = np.asarray(r.results[c]["outT"], np.float32).T
    return out



# revision 17
# speedup vs baseline: 1.4212x; 1.4212x over previous
"""Graphormer layer (pre-norm MHSA + additive attn bias + SiLU FFN) on 8 trn2 cores.

Sharding: core c handles batch b = c//4 and query rows i0 = (c%4)*512.
Each core computes LN1 + full K/V for its batch (replicated inside the
4-core batch group), Q/scores/softmax/attn@V for its 512 query rows, the
output projection, LN2 and the full FFN for those rows.  No collectives.

Host-side prep rotates each core's token axis by -i0 so the query block is
always columns 0:512 of the same SPMD program; the attn-bias j axis is
rotated identically (softmax/attn@V are order-invariant over j).

v2 vs v1:
  * All weights/bias/x arrive through ~46 large DMAs from host-packed
    [128, N] layouts (v1 used ~1200 x 32KB DMAs) and are kept resident, so
    Wk is loaded once instead of 4x.
  * attn bias is shipped as fp8 exp(bias)/16 and folded in with a bf16
    multiply after exp(scores) (softmax is invariant to the 1/16 scale);
    this halves bias DMA bytes and removes the f32 bias-add.
  * scores matmuls for the two heads sharing a kT tile are emitted as
    row-tiled pairs (partitions 0:64 / 64:128) so they overlap in the PE.
  * SiLU is a single ACT op; LN squares/copies run on the idle Pool
    (gpsimd) engine; rstd uses AF.Rsqrt.
  * LN1 / K / V are interleaved per 512-token block to keep PE fed and
    SBUF under budget.
"""

import sys
from contextlib import ExitStack

import numpy as np

sys.path.insert(0, "/opt/trn_rl_repo")

import ml_dtypes  # noqa: E402

import concourse.bass as bass  # noqa: E402
import concourse.bacc as bacc  # noqa: E402
import concourse.tile as tile  # noqa: E402
from concourse import mybir  # noqa: E402
from concourse.bass_utils import run_bass_kernel_spmd  # noqa: E402

F32 = mybir.dt.float32
BF16 = mybir.dt.bfloat16
F8 = mybir.dt.float8e4
AF = mybir.ActivationFunctionType
OP = mybir.AluOpType
BF16_NP = ml_dtypes.bfloat16
F8_NP = ml_dtypes.float8_e4m3

B, T, D = 2, 2048, 1024
H, HD = 16, 64
FF = 4 * D
N_CORES = 8
IB = 512           # query rows per core
SCALE = 1.0 / 8.0  # 1/sqrt(HD)
EPS = 1e-5
EB_SCALE = 1.0 / 16.0  # global scale on exp(bias); softmax-invariant

# CoreSim doesn't implement the Silu ACT table; simcheck.py flips this to
# use the (numerically equivalent) Sigmoid + DVE-multiply fallback.
SILU_ON_ACT = True

_cache = {}


def build_program():
    nc = bacc.Bacc("TRN2", target_bir_lowering=False, debug=False)

    # ---- DRAM I/O ----
    xT_d = nc.dram_tensor("xT", [D, T], F32, kind="ExternalInput").ap()
    ebT_d = nc.dram_tensor("ebT", [H, 128, 16 * IB], F8,
                           kind="ExternalInput").ap()
    Wq_d = nc.dram_tensor("Wq", [128, 8 * D], BF16, kind="ExternalInput").ap()
    Wk_d = nc.dram_tensor("Wk", [128, 8 * D], BF16, kind="ExternalInput").ap()
    Wv_d = nc.dram_tensor("Wv", [128, 8 * D], BF16, kind="ExternalInput").ap()
    Wo_d = nc.dram_tensor("Wo", [128, 8 * D], BF16, kind="ExternalInput").ap()
    W1_d = nc.dram_tensor("W1", [128, 8 * FF], BF16, kind="ExternalInput").ap()
    W2_d = nc.dram_tensor("W2", [128, 32 * D], BF16, kind="ExternalInput").ap()
    # packed per-partition params: [128, n_tiles] fp32
    g1_d = nc.dram_tensor("g1", [128, 8], F32, kind="ExternalInput").ap()
    bg1_d = nc.dram_tensor("bg1", [128, 8], F32, kind="ExternalInput").ap()
    g2_d = nc.dram_tensor("g2", [128, 8], F32, kind="ExternalInput").ap()
    bg2_d = nc.dram_tensor("bg2", [128, 8], F32, kind="ExternalInput").ap()
    bq8_d = nc.dram_tensor("bq8", [128, 8], F32, kind="ExternalInput").ap()
    bk_d = nc.dram_tensor("bk", [128, 8], F32, kind="ExternalInput").ap()
    bo_d = nc.dram_tensor("bo", [128, 8], F32, kind="ExternalInput").ap()
    b1_d = nc.dram_tensor("b1", [128, 32], F32, kind="ExternalInput").ap()
    b2_d = nc.dram_tensor("b2", [128, 8], F32, kind="ExternalInput").ap()
    bv_d = nc.dram_tensor("bv", [1, D], BF16, kind="ExternalInput").ap()
    outT_d = nc.dram_tensor("outT", [D, IB], F32, kind="ExternalOutput").ap()

    with tile.TileContext(nc) as tc, ExitStack() as ctx:
        # ---------------- outermost (whole-kernel lifetime) ----------------
        const_p = ctx.enter_context(tc.tile_pool(name="const", bufs=1))
        param_p = ctx.enter_context(tc.tile_pool(name="param", bufs=1))
        res_p = ctx.enter_context(tc.tile_pool(name="res", bufs=1))

        ones_f = const_p.tile([128, 2], F32, tag="ones_f")
        nc.vector.memset(ones_f[:], 1.0)
        ones_b = const_p.tile([1, 128], BF16, tag="ones_b")
        nc.vector.memset(ones_b[:], 1.0)
        eps_t = const_p.tile([1, 1], F32, tag="eps")
        nc.vector.memset(eps_t[:], EPS)

        def load_param(name, dram, shape, dtype=F32):
            t = param_p.tile(shape, dtype, tag=name, name=name)
            nc.sync.dma_start(t[:], dram[:])
            return t

        g1 = load_param("g1", g1_d, [128, 8])
        bg1 = load_param("bg1", bg1_d, [128, 8])
        g2 = load_param("g2", g2_d, [128, 8])
        bg2 = load_param("bg2", bg2_d, [128, 8])
        bq8 = load_param("bq8", bq8_d, [128, 8])
        bk = load_param("bk", bk_d, [128, 8])
        bo = load_param("bo", bo_d, [128, 8])
        b1 = load_param("b1", b1_d, [128, 32])
        b2 = load_param("b2", b2_d, [128, 8])
        bv = load_param("bv", bv_d, [1, D], BF16)

        res = [res_p.tile([128, IB], F32, tag=f"res{e}", name=f"res{e}")
               for e in range(8)]

        # ------- scope: K/V/Q + attention + out-proj (phases A-D1) ---------
        with tc.tile_pool(name="kT", bufs=1) as kT_p, \
             tc.tile_pool(name="vcat", bufs=1) as vcat_p, \
             tc.tile_pool(name="qT", bufs=1) as qT_p, \
             tc.tile_pool(name="oT", bufs=1) as oT_p:
            kT = [kT_p.tile([128, T], BF16, tag=f"kT{d}", name=f"kT{d}")
                  for d in range(8)]
            vcat = [vcat_p.tile([128, H * (HD + 1)], BF16, tag=f"vc{t}",
                                name=f"vc{t}") for t in range(16)]
            qT = [qT_p.tile([128, IB], BF16, tag=f"qT{d}", name=f"qT{d}")
                  for d in range(8)]
            oT = [oT_p.tile([128, IB], BF16, tag=f"oT{d}", name=f"oT{d}")
                  for d in range(8)]

            # ===== Phases A+B interleaved: LN1(n) -> [Q] -> K(n) -> V(n) ===
            with tc.tile_pool(name="wqkv", bufs=1) as wqkv_p, \
                 tc.tile_pool(name="xc", bufs=1) as xc_p, \
                 tc.tile_pool(name="hT", bufs=2) as hT_p, \
                 tc.tile_pool(name="sq", bufs=2) as sq_p, \
                 tc.tile_pool(name="lnt", bufs=1) as lnt_p, \
                 tc.tile_pool(name="lnb", bufs=1) as lnb_p, \
                 tc.tile_pool(name="lnps", bufs=2,
                              space=bass.MemorySpace.PSUM) as lnps_p, \
                 tc.tile_pool(name="pps", bufs=4,
                              space=bass.MemorySpace.PSUM) as pps:
                wq = wqkv_p.tile([128, 8 * D], BF16, tag="wq", name="wq")
                nc.sync.dma_start(wq[:], Wq_d[:])
                wk = wqkv_p.tile([128, 8 * D], BF16, tag="wk", name="wk")
                nc.sync.dma_start(wk[:], Wk_d[:])
                wv = wqkv_p.tile([128, 8 * D], BF16, tag="wv", name="wv")
                nc.sync.dma_start(wv[:], Wv_d[:])

                # ones columns of vcat (memset before V writes the rest)
                for tt in range(16):
                    nc.vector.memset(
                        vcat[tt][:].rearrange(
                            "p (h x) -> p h x", x=HD + 1)[:, :, HD:HD + 1],
                        1.0)

                for n in range(4):
                    nb = slice(n * 512, (n + 1) * 512)
                    # ---- LN1 stats for this 512-token block ----
                    xcs = []
                    ps_mu = lnps_p.tile([1, 512], F32, tag="psmu", name="psmu")
                    ps_sq = lnps_p.tile([1, 512], F32, tag="pssq", name="pssq")
                    for e in range(8):
                        xc = xc_p.tile([128, 512], F32, tag=f"xc{e}", name="xc")
                        nc.sync.dma_start(xc[:], xT_d[e * 128:(e + 1) * 128, nb])
                        xcs.append(xc)
                        nc.tensor.matmul(ps_mu[:], ones_f[:, 0:1], xc[:],
                                         start=(e == 0), stop=(e == 7))
                        x2 = sq_p.tile([128, 512], F32, tag="x2", name="x2")
                        nc.gpsimd.tensor_mul(x2[:], xc[:], xc[:])
                        nc.tensor.matmul(ps_sq[:], ones_f[:, 0:1], x2[:],
                                         start=(e == 0), stop=(e == 7))
                    mu_n = lnt_p.tile([1, 512], F32, tag="mu_n", name="mu_n")
                    nc.scalar.activation(mu_n[:], ps_mu[:], AF.Identity,
                                         scale=1.0 / D)
                    t1 = lnt_p.tile([1, 512], F32, tag="t1", name="t1")
                    nc.vector.tensor_mul(t1[:], mu_n[:], mu_n[:])
                    t2 = lnt_p.tile([1, 512], F32, tag="t2", name="t2")
                    nc.vector.scalar_tensor_tensor(
                        t2[:], ps_sq[:], 1.0 / D, t1[:],
                        op0=OP.mult, op1=OP.subtract)
                    nc.scalar.activation(t1[:], t2[:], AF.Sqrt, bias=eps_t[:])
                    nc.vector.reciprocal(t2[:], t1[:])
                    mu_b = lnb_p.tile([128, 512], F32, tag="mu_b", name="mu_b")
                    nc.gpsimd.partition_broadcast(mu_b[:], mu_n[:])
                    rstd_b = lnb_p.tile([128, 512], F32, tag="rstd_b",
                                        name="rstd_b")
                    nc.gpsimd.partition_broadcast(rstd_b[:], t2[:])
                    hTn = []
                    for e in range(8):
                        if n == 0:
                            nc.gpsimd.tensor_copy(res[e][:], xcs[e][:])
                        t = sq_p.tile([128, 512], F32, tag="lnap", name="lnap")
                        nc.vector.tensor_sub(t[:], xcs[e][:], mu_b[:])
                        nc.vector.tensor_mul(t[:], t[:], rstd_b[:])
                        h = hT_p.tile([128, 512], BF16, tag=f"hT{e}",
                                      name=f"hT{e}")
                        nc.scalar.activation(h[:], t[:], AF.Identity,
                                             scale=g1[:, e:e + 1],
                                             bias=bg1[:, e:e + 1])
                        hTn.append(h)

                    # ---- Q projection (query block == token cols 0:512) ----
                    if n == 0:
                        for dt in range(8):
                            ps = pps.tile([128, 512], F32, tag="ps", name="psq")
                            for e in range(8):
                                nc.tensor.matmul(
                                    ps[:],
                                    wq[:, e * D + dt * 128:e * D + (dt + 1) * 128],
                                    hTn[e][:], start=(e == 0), stop=(e == 7))
                            nc.scalar.activation(qT[dt][:], ps[:], AF.Identity,
                                                 scale=SCALE,
                                                 bias=bq8[:, dt:dt + 1])

                    # ---- K projection for this token block ----
                    for dt in range(8):
                        ps = pps.tile([128, 512], F32, tag="ps", name="psk")
                        for e in range(8):
                            nc.tensor.matmul(
                                ps[:],
                                wk[:, e * D + dt * 128:e * D + (dt + 1) * 128],
                                hTn[e][:], start=(e == 0), stop=(e == 7))
                        nc.scalar.activation(kT[dt][:, nb], ps[:], AF.Identity,
                                             bias=bk[:, dt:dt + 1])

                    # ---- V for this token block (both feature halves) ----
                    for tt in range(n * 4, n * 4 + 4):
                        tb = slice((tt - n * 4) * 128, (tt - n * 4 + 1) * 128)
                        for n2 in range(2):
                            n2b = slice(n2 * 512, (n2 + 1) * 512)
                            ps = pps.tile([128, 512], F32, tag="ps", name="psv")
                            for e in range(8):
                                nc.tensor.matmul(
                                    ps[:], hTn[e][:, tb],
                                    wv[:, e * D + n2 * 512:
                                       e * D + (n2 + 1) * 512],
                                    start=(e == 0), stop=False)
                            nc.tensor.matmul(ps[:], ones_b[:], bv[:, n2b],
                                             start=False, stop=True)
                            dst = vcat[tt][:, n2 * 8 * (HD + 1):
                                           (n2 + 1) * 8 * (HD + 1)]
                            dst = dst.rearrange("p (h x) -> p h x",
                                                x=HD + 1)[:, :, 0:HD]
                            src = ps[:].rearrange("p (h d) -> p h d", d=HD)
                            nc.scalar.activation(dst, src, AF.Identity)

            # ===== Phase C: attention (heads paired per kT tile) ===========
            with tc.tile_pool(name="wo", bufs=1) as wo_p, \
                 tc.tile_pool(name="ebias", bufs=2) as eb_p, \
                 tc.tile_pool(name="uexp", bufs=4) as ue_p, \
                 tc.tile_pool(name="umul", bufs=4) as u_p, \
                 tc.tile_pool(name="nrm", bufs=2) as nrm_p, \
                 tc.tile_pool(name="pss", bufs=4,
                              space=bass.MemorySpace.PSUM) as pss, \
                 tc.tile_pool(name="pso", bufs=4,
                              space=bass.MemorySpace.PSUM) as pso:
                wo = wo_p.tile([128, 8 * D], BF16, tag="wo", name="wo")
                nc.scalar.dma_start(wo[:], Wo_d[:])

                ebs = {}

                def load_eb(p):
                    t = eb_p.tile([128, 2 * 16 * IB], F8, tag="eb", name="eb")
                    nc.sync.dma_start(t[:], ebT_d[p])
                    ebs[p] = t

                load_eb(0)
                load_eb(1)
                for p in range(8):
                    eb = ebs.pop(p)
                    ps_o = [pso.tile([HD + 1, 512], F32, tag="ps_o",
                                     name="ps_o") for _ in range(2)]
                    for j in range(16):
                        jb = slice(j * 128, (j + 1) * 128)
                        us = []
                        for s in range(2):
                            po = s * 64
                            h = 2 * p + s
                            ps_s = pss.tile([128, 512], F32, tag="ps_s",
                                            name="ps_s")
                            nc.tensor.matmul(ps_s[:], kT[p][po:po + 64, jb],
                                             qT[p][po:po + 64, :],
                                             start=True, stop=True)
                            ue = ue_p.tile([128, IB], BF16, tag="ue", name="ue")
                            nc.scalar.activation(ue[:], ps_s[:], AF.Exp)
                            u = u_p.tile([128, IB], BF16, tag="u", name="u")
                            nc.vector.tensor_mul(
                                u[:], ue[:],
                                eb[:, (s * 16 + j) * IB:(s * 16 + j + 1) * IB])
                            us.append(u)
                        for s in range(2):
                            h = 2 * p + s
                            nc.tensor.matmul(
                                ps_o[s][:],
                                vcat[j][:, h * (HD + 1):(h + 1) * (HD + 1)],
                                us[s][:], start=(j == 0), stop=(j == 15))
                    if p + 2 < 8:
                        load_eb(p + 2)
                    for s in range(2):
                        po = s * 64
                        recip = nrm_p.tile([1, 512], F32, tag="recip",
                                           name="recip")
                        nc.vector.reciprocal(recip[:], ps_o[s][64:65, :])
                        rb = nrm_p.tile([64, 512], F32, tag="rb", name="rb")
                        nc.gpsimd.partition_broadcast(rb[:], recip[:])
                        nc.vector.tensor_mul(oT[p][po:po + 64, :],
                                             ps_o[s][0:64, :], rb[:])

                # ---- out-projection (reuses pss PSUM pool; inside C scope
                #      so wo/oT stay live) ----
                for et in range(8):
                    ps = pss.tile([128, 512], F32, tag="ps_s", name="psx1")
                    for dt in range(8):
                        nc.tensor.matmul(
                            ps[:],
                            wo[:, dt * D + et * 128:dt * D + (et + 1) * 128],
                            oT[dt][:], start=(dt == 0), stop=(dt == 7))
                    # res[et] <- x + attn_out (+bo), in place
                    nc.vector.scalar_tensor_tensor(res[et][:], ps[:],
                                                   bo[:, et:et + 1], res[et][:],
                                                   op0=OP.add, op1=OP.add)

        # ---------------- scope: LN2 + FFN (phases D2/E) -------------------
        with tc.tile_pool(name="h2", bufs=1) as h2_p, \
             tc.tile_pool(name="w1s", bufs=2) as w1s_p, \
             tc.tile_pool(name="w2s", bufs=2) as w2s_p, \
             tc.tile_pool(name="sz", bufs=1) as sz_p, \
             tc.tile_pool(name="out", bufs=2) as out_p, \
             tc.tile_pool(name="sq2", bufs=2) as sq2_p, \
             tc.tile_pool(name="lnt2", bufs=1) as lnt2_p, \
             tc.tile_pool(name="lnb2", bufs=1) as lnb2_p, \
             tc.tile_pool(name="dps1", bufs=2,
                          space=bass.MemorySpace.PSUM) as dps1, \
             tc.tile_pool(name="eps", bufs=4,
                          space=bass.MemorySpace.PSUM) as eps_p:
            w1cs = {}

            def load_w1(c):
                t = w1s_p.tile([128, 8192], BF16, tag="w1c", name=f"w1c{c}")
                nc.scalar.dma_start(t[:], W1_d[:, c * 8192:(c + 1) * 8192])
                w1cs[c] = t

            w2cs = {}

            def load_w2(c):
                t = w2s_p.tile([128, 8192], BF16, tag="w2c", name=f"w2c{c}")
                nc.scalar.dma_start(t[:], W2_d[:, c * 8192:(c + 1) * 8192])
                w2cs[c] = t

            load_w1(0)
            load_w1(1)

            h2 = [h2_p.tile([128, IB], BF16, tag=f"h2{e}", name=f"h2{e}")
                  for e in range(8)]

            # LN2 (single 512-col block)
            ps_mu = dps1.tile([1, 512], F32, tag="psmu2", name="psmu2")
            for e in range(8):
                nc.tensor.matmul(ps_mu[:], ones_f[:, 0:1], res[e][:],
                                 start=(e == 0), stop=(e == 7))
            ps_sq = dps1.tile([1, 512], F32, tag="pssq2", name="pssq2")
            for e in range(8):
                x2 = sq2_p.tile([128, 512], F32, tag="x22", name="x22")
                nc.gpsimd.tensor_mul(x2[:], res[e][:], res[e][:])
                nc.tensor.matmul(ps_sq[:], ones_f[:, 0:1], x2[:],
                                 start=(e == 0), stop=(e == 7))
            mu_n = lnt2_p.tile([1, 512], F32, tag="mu_n2", name="mu_n2")
            nc.scalar.activation(mu_n[:], ps_mu[:], AF.Identity, scale=1.0 / D)
            t1 = lnt2_p.tile([1, 512], F32, tag="t1_2", name="t1_2")
            nc.vector.tensor_mul(t1[:], mu_n[:], mu_n[:])
            t2 = lnt2_p.tile([1, 512], F32, tag="t2_2", name="t2_2")
            nc.vector.scalar_tensor_tensor(t2[:], ps_sq[:], 1.0 / D,
                                           t1[:], op0=OP.mult,
                                           op1=OP.subtract)
            nc.scalar.activation(t1[:], t2[:], AF.Sqrt, bias=eps_t[:])
            nc.vector.reciprocal(t2[:], t1[:])
            mu_b = lnb2_p.tile([128, 512], F32, tag="mu_b2", name="mu_b2")
            nc.gpsimd.partition_broadcast(mu_b[:], mu_n[:])
            rstd_b = lnb2_p.tile([128, 512], F32, tag="rstd_b2",
                                 name="rstd_b2")
            nc.gpsimd.partition_broadcast(rstd_b[:], t2[:])
            for e in range(8):
                t = sq2_p.tile([128, IB], F32, tag="lnap2", name="lnap2")
                nc.vector.tensor_sub(t[:], res[e][:], mu_b[:])
                nc.vector.tensor_mul(t[:], t[:], rstd_b[:])
                nc.scalar.activation(h2[e][:], t[:], AF.Identity,
                                     scale=g2[:, e:e + 1],
                                     bias=bg2[:, e:e + 1])

            # ===== Phase E: FFN ============================================
            sz = [sz_p.tile([128, IB], BF16, tag=f"sz{f}", name=f"sz{f}")
                  for f in range(32)]
            for ft in range(32):
                c = ft // 8
                if ft == 0:
                    load_w2(0)
                    load_w2(1)
                w1t = w1cs[c]
                ps = eps_p.tile([128, 512], F32, tag="pse", name="psz")
                for e in range(8):
                    nc.tensor.matmul(
                        ps[:],
                        w1t[:, (ft % 8) * 1024 + e * 128:
                            (ft % 8) * 1024 + (e + 1) * 128],
                        h2[e][:], start=(e == 0), stop=(e == 7))
                if SILU_ON_ACT:
                    nc.scalar.activation(sz[ft][:], ps[:], AF.Silu,
                                         bias=b1[:, ft:ft + 1])
                else:
                    sg = sq2_p.tile([128, IB], BF16, tag="sg", name="sg")
                    nc.scalar.activation(sg[:], ps[:], AF.Sigmoid,
                                         bias=b1[:, ft:ft + 1])
                    nc.vector.scalar_tensor_tensor(sz[ft][:], ps[:],
                                                   b1[:, ft:ft + 1], sg[:],
                                                   op0=OP.add, op1=OP.mult)
                if ft % 8 == 7 and c + 2 < 4:
                    load_w1(c + 2)
            for et in range(8):
                c = et // 2
                w2t = w2cs[c]
                ps = eps_p.tile([128, 512], F32, tag="pse", name="psy")
                for ft in range(32):
                    nc.tensor.matmul(
                        ps[:],
                        w2t[:, (et % 2) * 4096 + ft * 128:
                            (et % 2) * 4096 + (ft + 1) * 128],
                        sz[ft][:], start=(ft == 0), stop=(ft == 31))
                ot = out_p.tile([128, IB], F32, tag="outt", name="outt")
                nc.vector.scalar_tensor_tensor(ot[:], ps[:],
                                               b2[:, et:et + 1],
                                               res[et][:], op0=OP.add,
                                               op1=OP.add)
                nc.sync.dma_start(outT_d[et * 128:(et + 1) * 128, :], ot[:])
                if et % 2 == 1 and c + 2 < 4:
                    load_w2(c + 2)

    nc.compile()
    return nc


def _prep_inputs(inputs):
    """Host-side layout prep -> list of 8 per-core input maps."""
    x = np.asarray(inputs["x"], dtype=np.float32)
    ab = np.asarray(inputs["attn_bias"], dtype=np.float32)

    def pack(v, ntiles):
        return np.ascontiguousarray(
            np.asarray(v, np.float32).reshape(ntiles, 128).T)

    def packw(w, ntiles):  # [ntiles*128, N] -> [128, ntiles*N]
        w = np.asarray(w).astype(BF16_NP)
        n = w.shape[1]
        return np.ascontiguousarray(
            w.reshape(ntiles, 128, n).transpose(1, 0, 2).reshape(
                128, ntiles * n))

    # W1 is chunk-loaded in ft-major quarters: [p, ft*1024 + e*128 + j]
    w1 = np.asarray(inputs["W1"]).astype(BF16_NP)
    w1p = np.ascontiguousarray(
        w1.reshape(8, 128, 32, 128).transpose(1, 2, 0, 3).reshape(128, 32768))
    # W2 is chunk-loaded in et-major quarters: [p, et*4096 + ft*128 + j]
    w2 = np.asarray(inputs["W2"]).astype(BF16_NP)
    w2p = np.ascontiguousarray(
        w2.reshape(32, 128, 8, 128).transpose(1, 2, 0, 3).reshape(128, 32768))

    shared = {
        "Wq": packw(inputs["Wq"], 8),
        "Wk": packw(inputs["Wk"], 8),
        "Wv": packw(inputs["Wv"], 8),
        "Wo": packw(inputs["Wo"], 8),
        "W1": w1p,
        "W2": w2p,
        "g1": pack(inputs["ln1_g"], 8),
        "bg1": pack(inputs["ln1_b"], 8),
        "g2": pack(inputs["ln2_g"], 8),
        "bg2": pack(inputs["ln2_b"], 8),
        "bq8": pack(np.asarray(inputs["bq"], np.float32) * SCALE, 8),
        "bk": pack(inputs["bk"], 8),
        "bo": pack(inputs["bo"], 8),
        "b1": pack(inputs["b1"], 32),
        "b2": pack(inputs["b2"], 8),
        "bv": np.ascontiguousarray(
            np.asarray(inputs["bv"], np.float32).astype(BF16_NP).reshape(1, D)),
    }
    xT = [np.ascontiguousarray(x[b].T) for b in range(B)]  # [D, T] f32
    in_maps = []
    for c in range(N_CORES):
        b, i0 = c // 4, (c % 4) * IB
        # token axis rotated by -i0 (queries land at cols 0:IB); the j axis
        # of the bias is rotated identically to match k/v token order.
        xTc = np.ascontiguousarray(np.roll(xT[b], -i0, axis=1))
        # exp(bias)/16 as fp8, laid out [pair, 128, (s*16 + jblk)*512 + i]
        ebc = np.exp(ab[b, :, i0:i0 + IB, :]) * EB_SCALE      # [H, 512i, 2048j]
        ebc = np.roll(ebc, -i0, axis=2).transpose(0, 2, 1)    # [H, 2048j, 512i]
        ebc = ebc.reshape(H, 16, 128, IB).transpose(0, 2, 1, 3).reshape(
            H, 128, 16 * IB)                                  # [H, 128, 8192]
        ebc = ebc.reshape(H // 2, 2, 128, 16 * IB).transpose(
            0, 2, 1, 3).reshape(H // 2, 128, 2 * 16 * IB)     # [8, 128, 16384]
        m = {"xT": xTc, "ebT": np.ascontiguousarray(ebc.astype(F8_NP))}
        m.update(shared)
        in_maps.append(m)
    return in_maps


def kernel(**inputs):
    if "nc" not in _cache:
        _cache["nc"] = build_program()
    nc = _cache["nc"]
    in_maps = _prep_inputs(inputs)
    r = run_bass_kernel_spmd(nc, in_maps, list(range(N_CORES)))
    out = np.empty((B, T, D), dtype=np.float32)
    for c in range(N_CORES):
        b, i0 = c // 4, (c % 4) * IB
        out[b, i0:i0 + IB, :] = np.asarray(r.results[c]["outT"], np.float32).T
    return out


# revision 44
# speedup vs baseline: 1.8770x; 1.3206x over previous
"""Graphormer layer (pre-norm MHSA + additive attn bias + SiLU FFN) on 8 trn2 cores.

Sharding: core c handles batch b = c//4 and query rows i0 = (c%4)*512.
Each core computes LN1 + full K/V for its batch (replicated inside the
4-core batch group), Q/scores/softmax/attn@V for its 512 query rows, the
output projection, LN2 and the full FFN for those rows.  No collectives.

Host-side prep rotates each core's token axis by -i0 so the query block is
always columns 0:512 of the same SPMD program; the attn-bias j axis is
rotated identically (softmax/attn@V are order-invariant over j).

v2 vs v1:
  * All weights/bias/x arrive through ~46 large DMAs from host-packed
    [128, N] layouts (v1 used ~1200 x 32KB DMAs) and are kept resident, so
    Wk is loaded once instead of 4x.
  * attn bias is shipped as fp8 exp(bias)/16 and folded in with a bf16
    multiply after exp(scores) (softmax is invariant to the 1/16 scale);
    this halves bias DMA bytes and removes the f32 bias-add.
  * scores matmuls for the two heads sharing a kT tile are emitted as
    row-tiled pairs (partitions 0:64 / 64:128) so they overlap in the PE.
  * SiLU is a single ACT op; LN squares/copies run on the idle Pool
    (gpsimd) engine; rstd uses AF.Rsqrt.
  * LN1 / K / V are interleaved per 512-token block to keep PE fed and
    SBUF under budget.
"""

import sys
from contextlib import ExitStack

import numpy as np

sys.path.insert(0, "/opt/trn_rl_repo")

import ml_dtypes  # noqa: E402

import concourse.bass as bass  # noqa: E402
import concourse.bacc as bacc  # noqa: E402
import concourse.tile as tile  # noqa: E402
from concourse import mybir  # noqa: E402
from concourse.bass_utils import run_bass_kernel_spmd  # noqa: E402

F32 = mybir.dt.float32
F32R = mybir.dt.float32r
BF16 = mybir.dt.bfloat16
F8 = mybir.dt.float8e4
AF = mybir.ActivationFunctionType
OP = mybir.AluOpType
BF16_NP = ml_dtypes.bfloat16
F8_NP = ml_dtypes.float8_e4m3

B, T, D = 2, 2048, 1024
H, HD = 16, 64
FF = 4 * D
N_CORES = 8
IB = 512           # query rows per core
SCALE = 1.0 / 8.0  # 1/sqrt(HD)
EPS = 1e-5
EB_SCALE = 1.0 / 16.0  # global scale on exp(bias); softmax-invariant

# CoreSim doesn't implement the Silu ACT table; simcheck.py flips this to
# use the (numerically equivalent) Sigmoid + DVE-multiply fallback.
SILU_ON_ACT = True

_cache = {}


def build_program():
    nc = bacc.Bacc("TRN2", target_bir_lowering=False, debug=False)

    # ---- DRAM I/O ----
    xT_d = nc.dram_tensor("xT", [D, T], F32, kind="ExternalInput").ap()
    xbT_d = nc.dram_tensor("xb", [D, T], BF16, kind="ExternalInput").ap()
    ebT_d = nc.dram_tensor("ebT", [H, 128, 16 * IB], F8,
                           kind="ExternalInput").ap()
    Wq_d = nc.dram_tensor("Wq", [128, 8 * D], BF16, kind="ExternalInput").ap()
    Wk_d = nc.dram_tensor("Wk", [128, 8 * D], BF16, kind="ExternalInput").ap()
    Wv_d = nc.dram_tensor("Wv", [128, 8 * D], BF16, kind="ExternalInput").ap()
    Wo_d = nc.dram_tensor("Wo", [128, 8 * D], BF16, kind="ExternalInput").ap()
    W1_d = nc.dram_tensor("W1", [128, 8 * FF], BF16, kind="ExternalInput").ap()
    W2_d = nc.dram_tensor("W2", [128, 32 * D], BF16, kind="ExternalInput").ap()
    # packed per-partition params, one DMA: [128, 96] fp32
    # cols: g1 bg1 g2 bg2 bq8 bk bo b2 (8 each), then b1 (32)
    par_d = nc.dram_tensor("par", [128, 96], F32, kind="ExternalInput").ap()
    bv_d = nc.dram_tensor("bv", [1, D], BF16, kind="ExternalInput").ap()
    outT_d = nc.dram_tensor("outT", [D, IB], F32, kind="ExternalOutput").ap()

    with tile.TileContext(nc) as tc, ExitStack() as ctx:
        # ---------------- outermost (whole-kernel lifetime) ----------------
        const_p = ctx.enter_context(tc.tile_pool(name="const", bufs=1))
        param_p = ctx.enter_context(tc.tile_pool(name="param", bufs=1))
        res_p = ctx.enter_context(tc.tile_pool(name="res", bufs=1))

        ones_f = const_p.tile([128, 2], F32, tag="ones_f")
        nc.vector.memset(ones_f[:], 1.0)
        ones_c = const_p.tile([128, 1], BF16, tag="ones_c")
        nc.vector.memset(ones_c[:], 1.0)
        ones_b = const_p.tile([1, 128], BF16, tag="ones_b")
        nc.vector.memset(ones_b[:], 1.0)
        eps_t = const_p.tile([1, 1], F32, tag="eps")
        nc.vector.memset(eps_t[:], EPS)

        par = param_p.tile([128, 96], F32, tag="par", name="par")
        nc.scalar.dma_start(par[:], par_d[:])
        g1, bg1, g2, bg2 = par[:, 0:8], par[:, 8:16], par[:, 16:24], \
            par[:, 24:32]
        bq8, bk, bo, b2 = par[:, 32:40], par[:, 40:48], par[:, 48:56], \
            par[:, 56:64]
        b1 = par[:, 64:96]
        bv = param_p.tile([1, D], BF16, tag="bv", name="bv")
        nc.scalar.dma_start(bv[:], bv_d[:])
        # bv broadcast across partitions (added to V tiles with a DVE add)
        bvb = param_p.tile([128, D], BF16, tag="bvb", name="bvb")
        nc.gpsimd.partition_broadcast(bvb[:], bv[:])

        res = [res_p.tile([128, IB], F32, tag=f"res{e}", name=f"res{e}")
               for e in range(8)]

        # ------- scope: K/V/Q + attention + out-proj (phases A-D1) ---------
        with tc.tile_pool(name="kT", bufs=1) as kT_p, \
             tc.tile_pool(name="vcat", bufs=1) as vcat_p, \
             tc.tile_pool(name="qT", bufs=1) as qT_p:
            kT = [kT_p.tile([128, T], BF16, tag=f"kT{d}", name=f"kT{d}")
                  for d in range(8)]
            vcat = [vcat_p.tile([128, H * (HD + 1)], BF16, tag=f"vc{t}",
                                name=f"vc{t}") for t in range(16)]
            qT = [qT_p.tile([128, IB], BF16, tag=f"qT{d}", name=f"qT{d}")
                  for d in range(8)]

            # ===== Phases A+B interleaved: LN1(n) -> [Q] -> K(n) -> V(n) ===
            # LN stats run one block ahead so the mu/rstd chain overlaps the
            # previous block's K/V matmuls.
            with tc.tile_pool(name="wqkv", bufs=1) as wqkv_p, \
                 tc.tile_pool(name="xc", bufs=2) as xc_p, \
                 tc.tile_pool(name="hT", bufs=2) as hT_p, \
                 tc.tile_pool(name="sq", bufs=2) as sq_p, \
                 tc.tile_pool(name="lnt", bufs=1) as lnt_p, \
                 tc.tile_pool(name="lnb", bufs=2) as lnb_p, \
                 tc.tile_pool(name="lnps", bufs=2,
                              space=bass.MemorySpace.PSUM) as lnps_p, \
                 tc.tile_pool(name="pps", bufs=4,
                              space=bass.MemorySpace.PSUM) as pps:
                # ones columns of vcat (memset before V writes the rest)
                for tt in range(16):
                    nc.vector.memset(
                        vcat[tt][:].rearrange(
                            "p (h x) -> p h x", x=HD + 1)[:, :, HD:HD + 1],
                        1.0)

                wqkv = {}

                def load_w(nm, dram):
                    t = wqkv_p.tile([128, 8 * D], BF16, tag=nm, name=nm)
                    nc.sync.dma_start(t[:], dram[:])
                    wqkv[nm] = t

                def ln_stats(n):
                    nb = slice(n * 512, (n + 1) * 512)
                    xcs, stats = [], {}
                    ps_mu = lnps_p.tile([1, 512], F32, tag="psmu", name="psmu")
                    ps_sq = lnps_p.tile([1, 512], F32, tag="pssq", name="pssq")
                    for e in range(8):
                        xc = xc_p.tile([128, 512], BF16, tag=f"xc{e}",
                                       name="xc")
                        nc.sync.dma_start(xc[:],
                                          xbT_d[e * 128:(e + 1) * 128, nb])
                        xcs.append(xc)
                        nc.tensor.matmul(ps_mu[:], ones_c[:], xc[:],
                                         start=(e == 0), stop=(e == 7))
                        x2 = sq_p.tile([128, 512], BF16, tag="x2", name="x2")
                        nc.vector.tensor_mul(x2[:], xc[:], xc[:])
                        nc.tensor.matmul(ps_sq[:], ones_c[:], x2[:],
                                         start=(e == 0), stop=(e == 7))
                    stats["mu"], stats["sq"], stats["xcs"] = ps_mu, ps_sq, xcs
                    return stats

                def ln_normalize(st):
                    mu_n = lnt_p.tile([1, 512], F32, tag="mu_n", name="mu_n")
                    nc.scalar.activation(mu_n[:], st["mu"][:], AF.Identity,
                                         scale=1.0 / D)
                    t1 = lnt_p.tile([1, 512], F32, tag="t1", name="t1")
                    nc.vector.tensor_mul(t1[:], mu_n[:], mu_n[:])
                    t2 = lnt_p.tile([1, 512], F32, tag="t2", name="t2")
                    nc.vector.scalar_tensor_tensor(
                        t2[:], st["sq"][:], 1.0 / D, t1[:],
                        op0=OP.mult, op1=OP.subtract)
                    nc.scalar.activation(t1[:], t2[:], AF.Sqrt, bias=eps_t[:])
                    nc.vector.reciprocal(t2[:], t1[:])
                    mu_b = lnb_p.tile([128, 512], F32, tag="mu_b", name="mu_b")
                    nc.gpsimd.partition_broadcast(mu_b[:], mu_n[:])
                    rstd_b = lnb_p.tile([128, 512], F32, tag="rstd_b",
                                        name="rstd_b")
                    nc.gpsimd.partition_broadcast(rstd_b[:], t2[:])
                    hTn = []
                    for e in range(8):
                        t = sq_p.tile([128, 512], F32, tag="lnap", name="lnap")
                        nc.vector.tensor_sub(t[:], st["xcs"][e][:], mu_b[:])
                        nc.vector.tensor_mul(t[:], t[:], rstd_b[:])
                        h = hT_p.tile([128, 512], BF16, tag=f"hT{e}",
                                      name=f"hT{e}")
                        nc.scalar.activation(h[:], t[:], AF.Identity,
                                             scale=g1[:, e:e + 1],
                                             bias=bg1[:, e:e + 1])
                        hTn.append(h)
                    return hTn

                st = ln_stats(0)
                load_w("wq", Wq_d)
                st_next = ln_stats(1)
                load_w("wk", Wk_d)
                load_w("wv", Wv_d)
                for n in range(4):
                    nb = slice(n * 512, (n + 1) * 512)
                    hTn = ln_normalize(st)
                    st = st_next
                    if n + 2 < 4:
                        st_next = ln_stats(n + 2)

                    # ---- Q projection (query block == token cols 0:512) ----
                    if n == 0:
                        wq = wqkv["wq"]
                        for dt in range(8):
                            ps = pps.tile([128, 512], F32, tag="ps", name="psq")
                            for e in range(8):
                                nc.tensor.matmul(
                                    ps[:],
                                    wq[:, e * D + dt * 128:e * D + (dt + 1) * 128],
                                    hTn[e][:], start=(e == 0), stop=(e == 7))
                            nc.scalar.activation(qT[dt][:], ps[:], AF.Identity,
                                                 scale=SCALE,
                                                 bias=bq8[:, dt:dt + 1])

                    # ---- K projection for this token block ----
                    wk = wqkv["wk"]
                    for dt in range(8):
                        ps = pps.tile([128, 512], F32, tag="ps", name="psk")
                        for e in range(8):
                            nc.tensor.matmul(
                                ps[:],
                                wk[:, e * D + dt * 128:e * D + (dt + 1) * 128],
                                hTn[e][:], start=(e == 0), stop=(e == 7))
                        nc.vector.tensor_scalar_add(kT[dt][:, nb], ps[:],
                                                    bk[:, dt:dt + 1])

                    # ---- V for this token block (both feature halves) ----
                    wv = wqkv["wv"]
                    for tt in range(n * 4, n * 4 + 4):
                        tb = slice((tt - n * 4) * 128, (tt - n * 4 + 1) * 128)
                        for n2 in range(2):
                            n2b = slice(n2 * 512, (n2 + 1) * 512)
                            ps = pps.tile([128, 512], F32, tag="ps", name="psv")
                            for e in range(8):
                                nc.tensor.matmul(
                                    ps[:], hTn[e][:, tb],
                                    wv[:, e * D + n2 * 512:
                                       e * D + (n2 + 1) * 512],
                                    start=(e == 0), stop=(e == 7))
                            dst = vcat[tt][:, n2 * 8 * (HD + 1):
                                           (n2 + 1) * 8 * (HD + 1)]
                            dst = dst.rearrange("p (h x) -> p h x",
                                                x=HD + 1)[:, :, 0:HD]
                            src = ps[:].rearrange("p (h d) -> p h d", d=HD)
                            bvs = bvb[:, n2b].rearrange("p (h d) -> p h d",
                                                        d=HD)
                            nc.vector.tensor_add(dst, src, bvs)

            # ===== Phase C: attention (heads paired per kT tile) ===========
            # scores for both heads of a pair land in one [128,1024] PSUM
            # tile (2 banks); exp is one fused ACT op; the exp(bias) multiply
            # is split across DVE (head 0) and Pool (head 1).
            with tc.tile_pool(name="wo", bufs=1) as wo_p, \
                 tc.tile_pool(name="oT", bufs=1) as oT_p, \
                 tc.tile_pool(name="ebias", bufs=3) as eb_p, \
                 tc.tile_pool(name="uexp", bufs=3) as ue_p, \
                 tc.tile_pool(name="umul", bufs=3) as u_p, \
                 tc.tile_pool(name="nrm", bufs=2) as nrm_p, \
                 tc.tile_pool(name="pss", bufs=2,
                              space=bass.MemorySpace.PSUM) as pss, \
                 tc.tile_pool(name="pso", bufs=4,
                              space=bass.MemorySpace.PSUM) as pso:
                oT = [oT_p.tile([128, IB], BF16, tag=f"oT{d}", name=f"oT{d}")
                      for d in range(8)]
                wo = wo_p.tile([128, 8 * D], BF16, tag="wo", name="wo")

                ebs = {}

                def load_eb(h):
                    t = eb_p.tile([128, 16 * IB], F8, tag="eb", name=f"eb{h}")
                    nc.sync.dma_start(t[:], ebT_d[h])
                    ebs[h] = t

                load_eb(0)
                load_eb(1)
                load_eb(2)
                for p in range(8):
                    eb0, eb1 = ebs.pop(2 * p), ebs.pop(2 * p + 1)
                    ps_o = [pso.tile([HD + 1, 512], F32, tag="ps_o",
                                     name="ps_o") for _ in range(2)]
                    us = [None] * 16

                    def attn_av(j):
                        # attn@V for block j (emitted one step behind the
                        # scores/exp chain so the PE FIFO never blocks on
                        # the current block's exp)
                        for s in range(2):
                            h = 2 * p + s
                            nc.tensor.matmul(
                                ps_o[s][:],
                                vcat[j][:, h * (HD + 1):(h + 1) * (HD + 1)],
                                us[j][:, s * 512:(s + 1) * 512],
                                start=(j == 0), stop=(j == 15))

                    for j in range(16):
                        jb = slice(j * 128, (j + 1) * 128)
                        ps_s = pss.tile([128, 1024], F32, tag="ps_s",
                                        name="ps_s")
                        for s in range(2):
                            po = s * 64
                            nc.tensor.matmul(ps_s[:, s * 512:(s + 1) * 512],
                                             kT[p][po:po + 64, jb],
                                             qT[p][po:po + 64, :],
                                             start=True, stop=True)
                        ue = ue_p.tile([128, 1024], BF16, tag="ue", name="ue")
                        nc.scalar.activation(ue[:], ps_s[:], AF.Exp)
                        u = u_p.tile([128, 1024], BF16, tag="u", name="u")
                        nc.vector.tensor_mul(u[:, 0:512], ue[:, 0:512],
                                             eb0[:, j * IB:(j + 1) * IB])
                        nc.gpsimd.tensor_mul(u[:, 512:1024], ue[:, 512:1024],
                                             eb1[:, j * IB:(j + 1) * IB])
                        us[j] = u
                        if j > 0:
                            attn_av(j - 1)
                    attn_av(15)
                    if 2 * p + 3 < H:
                        load_eb(2 * p + 3)
                    for s in range(2):
                        po = s * 64
                        recip = nrm_p.tile([1, 512], F32, tag="recip",
                                           name="recip")
                        nc.vector.reciprocal_approx_accurate(recip[:],
                                                             ps_o[s][64:65, :])
                        rb = nrm_p.tile([64, 512], F32, tag="rb", name="rb")
                        nc.gpsimd.partition_broadcast(rb[:], recip[:])
                        nc.vector.tensor_mul(oT[p][po:po + 64, :],
                                             ps_o[s][0:64, :], rb[:])
                    if 2 * p + 4 < H:
                        load_eb(2 * p + 4)
                    if p == 0:
                        # x residual rows for the query block (for out-proj)
                        for e in range(8):
                            nc.sync.dma_start(res[e][:],
                                              xT_d[e * 128:(e + 1) * 128, 0:IB])
                    elif p == 1:
                        nc.sync.dma_start(wo[:], Wo_d[:])

                # ---- out-projection (reuses pss PSUM pool; inside C scope
                #      so wo/oT stay live) ----
                for et in range(8):
                    ps = pss.tile([128, 1024], F32, tag="ps_s", name="psx1")
                    for dt in range(8):
                        nc.tensor.matmul(
                            ps[:, 0:512],
                            wo[:, dt * D + et * 128:dt * D + (et + 1) * 128],
                            oT[dt][:], start=(dt == 0), stop=(dt == 7))
                    # res[et] <- x + attn_out (+bo), in place
                    nc.vector.scalar_tensor_tensor(res[et][:], ps[:, 0:512],
                                                   bo[:, et:et + 1], res[et][:],
                                                   op0=OP.add, op1=OP.add)

        # ---------------- scope: LN2 + FFN (phases D2/E) -------------------
        with tc.tile_pool(name="h2", bufs=1) as h2_p, \
             tc.tile_pool(name="w1s", bufs=2) as w1s_p, \
             tc.tile_pool(name="w2s", bufs=2) as w2s_p, \
             tc.tile_pool(name="sz", bufs=1) as sz_p, \
             tc.tile_pool(name="out", bufs=2) as out_p, \
             tc.tile_pool(name="sq2", bufs=2) as sq2_p, \
             tc.tile_pool(name="lnt2", bufs=1) as lnt2_p, \
             tc.tile_pool(name="lnb2", bufs=1) as lnb2_p, \
             tc.tile_pool(name="dps1", bufs=2,
                          space=bass.MemorySpace.PSUM) as dps1, \
             tc.tile_pool(name="eps", bufs=4,
                          space=bass.MemorySpace.PSUM) as eps_p:
            w1cs = {}

            def load_w1(c):
                t = w1s_p.tile([128, 8192], BF16, tag="w1c", name=f"w1c{c}")
                nc.sync.dma_start(t[:], W1_d[:, c * 8192:(c + 1) * 8192])
                w1cs[c] = t

            w2cs = {}

            def load_w2(c):
                t = w2s_p.tile([128, 8192], BF16, tag="w2c", name=f"w2c{c}")
                nc.sync.dma_start(t[:], W2_d[:, c * 8192:(c + 1) * 8192])
                w2cs[c] = t

            load_w1(0)
            load_w1(1)

            h2 = [h2_p.tile([128, IB], BF16, tag=f"h2{e}", name=f"h2{e}")
                  for e in range(8)]

            # LN2 (single 512-col block)
            ps_mu = dps1.tile([1, 512], F32, tag="psmu2", name="psmu2")
            for e in range(8):
                xb = sq2_p.tile([128, 512], BF16, tag="xb2", name="xb2")
                nc.gpsimd.tensor_copy(xb[:], res[e][:])
                nc.tensor.matmul(ps_mu[:], ones_c[:], xb[:],
                                 start=(e == 0), stop=(e == 7))
            ps_sq = dps1.tile([1, 512], F32, tag="pssq2", name="pssq2")
            for e in range(8):
                x2 = sq2_p.tile([128, 512], BF16, tag="x22", name="x22")
                nc.gpsimd.tensor_mul(x2[:], res[e][:], res[e][:])
                nc.tensor.matmul(ps_sq[:], ones_c[:], x2[:],
                                 start=(e == 0), stop=(e == 7))
            mu_n = lnt2_p.tile([1, 512], F32, tag="mu_n2", name="mu_n2")
            nc.scalar.activation(mu_n[:], ps_mu[:], AF.Identity, scale=1.0 / D)
            t1 = lnt2_p.tile([1, 512], F32, tag="t1_2", name="t1_2")
            nc.vector.tensor_mul(t1[:], mu_n[:], mu_n[:])
            t2 = lnt2_p.tile([1, 512], F32, tag="t2_2", name="t2_2")
            nc.vector.scalar_tensor_tensor(t2[:], ps_sq[:], 1.0 / D,
                                           t1[:], op0=OP.mult,
                                           op1=OP.subtract)
            nc.scalar.activation(t1[:], t2[:], AF.Sqrt, bias=eps_t[:])
            nc.vector.reciprocal(t2[:], t1[:])
            mu_b = lnb2_p.tile([128, 512], F32, tag="mu_b2", name="mu_b2")
            nc.gpsimd.partition_broadcast(mu_b[:], mu_n[:])
            rstd_b = lnb2_p.tile([128, 512], F32, tag="rstd_b2",
                                 name="rstd_b2")
            nc.gpsimd.partition_broadcast(rstd_b[:], t2[:])
            for e in range(8):
                t = sq2_p.tile([128, IB], F32, tag="lnap2", name="lnap2")
                nc.vector.tensor_sub(t[:], res[e][:], mu_b[:])
                nc.vector.tensor_mul(t[:], t[:], rstd_b[:])
                nc.scalar.activation(h2[e][:], t[:], AF.Identity,
                                     scale=g2[:, e:e + 1],
                                     bias=bg2[:, e:e + 1])

            # ===== Phase E: FFN ============================================
            sz = [sz_p.tile([128, IB], BF16, tag=f"sz{f}", name=f"sz{f}")
                  for f in range(32)]
            for ft in range(32):
                c = ft // 8
                if ft == 0:
                    load_w2(0)
                    load_w2(1)
                w1t = w1cs[c]
                ps = eps_p.tile([128, 512], F32, tag="pse", name="psz")
                for e in range(8):
                    nc.tensor.matmul(
                        ps[:],
                        w1t[:, (ft % 8) * 1024 + e * 128:
                            (ft % 8) * 1024 + (e + 1) * 128],
                        h2[e][:], start=(e == 0), stop=(e == 7))
                if SILU_ON_ACT:
                    nc.scalar.activation(sz[ft][:], ps[:], AF.Silu,
                                         bias=b1[:, ft:ft + 1])
                else:
                    sg = sq2_p.tile([128, IB], BF16, tag="sg", name="sg")
                    nc.scalar.activation(sg[:], ps[:], AF.Sigmoid,
                                         bias=b1[:, ft:ft + 1])
                    nc.vector.scalar_tensor_tensor(sz[ft][:], ps[:],
                                                   b1[:, ft:ft + 1], sg[:],
                                                   op0=OP.add, op1=OP.mult)
                if ft % 8 == 7 and c + 2 < 4:
                    load_w1(c + 2)
            for et in range(8):
                c = et // 2
                w2t = w2cs[c]
                ps = eps_p.tile([128, 512], F32, tag="pse", name="psy")
                for ft in range(32):
                    nc.tensor.matmul(
                        ps[:],
                        w2t[:, (et % 2) * 4096 + ft * 128:
                            (et % 2) * 4096 + (ft + 1) * 128],
                        sz[ft][:], start=(ft == 0), stop=(ft == 31))
                ot = out_p.tile([128, IB], F32, tag="outt", name="outt")
                nc.vector.scalar_tensor_tensor(ot[:], ps[:],
                                               b2[:, et:et + 1],
                                               res[et][:], op0=OP.add,
                                               op1=OP.add)
                nc.sync.dma_start(outT_d[et * 128:(et + 1) * 128, :], ot[:])
                if et % 2 == 1 and c + 2 < 4:
                    load_w2(c + 2)

    nc.compile()
    return nc


def _prep_inputs(inputs):
    """Host-side layout prep -> list of 8 per-core input maps."""
    x = np.asarray(inputs["x"], dtype=np.float32)
    ab = np.asarray(inputs["attn_bias"], dtype=np.float32)

    def pack(v, ntiles):
        return np.ascontiguousarray(
            np.asarray(v, np.float32).reshape(ntiles, 128).T)

    def packw(w, ntiles):  # [ntiles*128, N] -> [128, ntiles*N]
        w = np.asarray(w).astype(BF16_NP)
        n = w.shape[1]
        return np.ascontiguousarray(
            w.reshape(ntiles, 128, n).transpose(1, 0, 2).reshape(
                128, ntiles * n))

    # W1 is chunk-loaded in ft-major quarters: [p, ft*1024 + e*128 + j]
    w1 = np.asarray(inputs["W1"]).astype(BF16_NP)
    w1p = np.ascontiguousarray(
        w1.reshape(8, 128, 32, 128).transpose(1, 2, 0, 3).reshape(128, 32768))
    # W2 is chunk-loaded in et-major quarters: [p, et*4096 + ft*128 + j]
    w2 = np.asarray(inputs["W2"]).astype(BF16_NP)
    w2p = np.ascontiguousarray(
        w2.reshape(32, 128, 8, 128).transpose(1, 2, 0, 3).reshape(128, 32768))

    par = np.concatenate([
        pack(inputs["ln1_g"], 8), pack(inputs["ln1_b"], 8),
        pack(inputs["ln2_g"], 8), pack(inputs["ln2_b"], 8),
        pack(np.asarray(inputs["bq"], np.float32) * SCALE, 8),
        pack(inputs["bk"], 8), pack(inputs["bo"], 8), pack(inputs["b2"], 8),
        pack(inputs["b1"], 32)], axis=1)

    shared = {
        "Wq": packw(inputs["Wq"], 8),
        "Wk": packw(inputs["Wk"], 8),
        "Wv": packw(inputs["Wv"], 8),
        "Wo": packw(inputs["Wo"], 8),
        "W1": w1p,
        "W2": w2p,
        "par": np.ascontiguousarray(par),
        "bv": np.ascontiguousarray(
            np.asarray(inputs["bv"], np.float32).astype(BF16_NP).reshape(1, D)),
    }
    xT = [np.ascontiguousarray(x[b].T) for b in range(B)]  # [D, T] f32
    in_maps = []
    for c in range(N_CORES):
        b, i0 = c // 4, (c % 4) * IB
        # token axis rotated by -i0 (queries land at cols 0:IB); the j axis
        # of the bias is rotated identically to match k/v token order.
        xTc = np.ascontiguousarray(np.roll(xT[b], -i0, axis=1))
        # exp(bias)/16 as fp8, laid out [pair, 128, (s*16 + jblk)*512 + i]
        ebc = np.exp(ab[b, :, i0:i0 + IB, :]) * EB_SCALE      # [H, 512i, 2048j]
        ebc = np.roll(ebc, -i0, axis=2).transpose(0, 2, 1)    # [H, 2048j, 512i]
        ebc = ebc.reshape(H, 16, 128, IB).transpose(0, 2, 1, 3).reshape(
            H, 128, 16 * IB)                                  # [H, 128, 8192]
        m = {"xT": xTc, "xb": np.ascontiguousarray(xTc.astype(BF16_NP)),
             "ebT": np.ascontiguousarray(ebc.astype(F8_NP))}
        m.update(shared)
        in_maps.append(m)
    return in_maps


def kernel(**inputs):
    if "nc" not in _cache:
        _cache["nc"] = build_program()
    nc = _cache["nc"]
    in_maps = _prep_inputs(inputs)
    r = run_bass_kernel_spmd(nc, in_maps, list(range(N_CORES)))
    out = np.empty((B, T, D), dtype=np.float32)
    for c in range(N_CORES):
        b, i0 = c // 4, (c % 4) * IB
        out[b, i0:i0 + IB, :] = np.asarray(r.results[c]["outT"], np.float32).T
    return out


# revision 46
# speedup vs baseline: 2.0124x; 1.0721x over previous
"""Graphormer layer (pre-norm MHSA + additive attn bias + SiLU FFN) on 8 trn2 cores.

Sharding: core c handles batch b = c//4 and query rows i0 = (c%4)*512.
Each core computes LN1 + full K/V for its batch (replicated inside the
4-core batch group), Q/scores/softmax/attn@V for its 512 query rows, the
output projection, LN2 and the full FFN for those rows.  No collectives.

Host-side prep rotates each core's token axis by -i0 so the query block is
always columns 0:512 of the same SPMD program; the attn-bias j axis is
rotated identically (softmax/attn@V are order-invariant over j).

v2 vs v1:
  * All weights/bias/x arrive through ~46 large DMAs from host-packed
    [128, N] layouts (v1 used ~1200 x 32KB DMAs) and are kept resident, so
    Wk is loaded once instead of 4x.
  * attn bias is shipped as fp8 exp(bias)/16 and folded in with a bf16
    multiply after exp(scores) (softmax is invariant to the 1/16 scale);
    this halves bias DMA bytes and removes the f32 bias-add.
  * scores matmuls for the two heads sharing a kT tile are emitted as
    row-tiled pairs (partitions 0:64 / 64:128) so they overlap in the PE.
  * SiLU is a single ACT op; LN squares/copies run on the idle Pool
    (gpsimd) engine; rstd uses AF.Rsqrt.
  * LN1 / K / V are interleaved per 512-token block to keep PE fed and
    SBUF under budget.
"""

import sys
from contextlib import ExitStack

import numpy as np

sys.path.insert(0, "/opt/trn_rl_repo")

import ml_dtypes  # noqa: E402

import concourse.bass as bass  # noqa: E402
import concourse.bacc as bacc  # noqa: E402
import concourse.tile as tile  # noqa: E402
from concourse import mybir  # noqa: E402
from concourse.bass_utils import run_bass_kernel_spmd  # noqa: E402

F32 = mybir.dt.float32
F32R = mybir.dt.float32r
BF16 = mybir.dt.bfloat16
F8 = mybir.dt.float8e4
AF = mybir.ActivationFunctionType
OP = mybir.AluOpType
BF16_NP = ml_dtypes.bfloat16
F8_NP = ml_dtypes.float8_e4m3

B, T, D = 2, 2048, 1024
H, HD = 16, 64
FF = 4 * D
N_CORES = 8
IB = 512           # query rows per core
SCALE = 1.0 / 8.0  # 1/sqrt(HD)
EPS = 1e-5
EB_SCALE = 1.0 / 16.0  # global scale on exp(bias); softmax-invariant

# CoreSim doesn't implement the Silu ACT table; simcheck.py flips this to
# use the (numerically equivalent) Sigmoid + DVE-multiply fallback.
SILU_ON_ACT = True

_cache = {}


def build_program():
    nc = bacc.Bacc("TRN2", target_bir_lowering=False, debug=False)

    # ---- DRAM I/O ----
    xT_d = nc.dram_tensor("xT", [D, T], F32, kind="ExternalInput").ap()
    xbT_d = nc.dram_tensor("xb", [D, T], BF16, kind="ExternalInput").ap()
    ebT_d = nc.dram_tensor("ebT", [H, 128, 16 * IB], F8,
                           kind="ExternalInput").ap()
    Wq_d = nc.dram_tensor("Wq", [128, 8 * D], BF16, kind="ExternalInput").ap()
    Wk_d = nc.dram_tensor("Wk", [128, 8 * D], BF16, kind="ExternalInput").ap()
    Wv_d = nc.dram_tensor("Wv", [128, 8 * D], BF16, kind="ExternalInput").ap()
    Wo_d = nc.dram_tensor("Wo", [128, 8 * D], BF16, kind="ExternalInput").ap()
    W1_d = nc.dram_tensor("W1", [128, 8 * FF], BF16, kind="ExternalInput").ap()
    W2_d = nc.dram_tensor("W2", [128, 32 * D], BF16, kind="ExternalInput").ap()
    # packed per-partition params, one DMA: [128, 96] fp32
    # cols: g1 bg1 g2 bg2 bq8 bk bo b2 (8 each), then b1 (32)
    par_d = nc.dram_tensor("par", [128, 96], F32, kind="ExternalInput").ap()
    bv_d = nc.dram_tensor("bv", [1, D], BF16, kind="ExternalInput").ap()
    outT_d = nc.dram_tensor("outT", [D, IB], F32, kind="ExternalOutput").ap()

    with tile.TileContext(nc) as tc, ExitStack() as ctx:
        # ---------------- outermost (whole-kernel lifetime) ----------------
        const_p = ctx.enter_context(tc.tile_pool(name="const", bufs=1))
        param_p = ctx.enter_context(tc.tile_pool(name="param", bufs=1))
        res_p = ctx.enter_context(tc.tile_pool(name="res", bufs=1))

        ones_f = const_p.tile([128, 2], F32, tag="ones_f")
        nc.vector.memset(ones_f[:], 1.0)
        ones_c = const_p.tile([128, 1], BF16, tag="ones_c")
        nc.vector.memset(ones_c[:], 1.0)
        ones_b = const_p.tile([1, 128], BF16, tag="ones_b")
        nc.vector.memset(ones_b[:], 1.0)
        eps_t = const_p.tile([1, 1], F32, tag="eps")
        nc.vector.memset(eps_t[:], EPS)

        par = param_p.tile([128, 96], F32, tag="par", name="par")
        nc.scalar.dma_start(par[:], par_d[:])
        g1, bg1, g2, bg2 = par[:, 0:8], par[:, 8:16], par[:, 16:24], \
            par[:, 24:32]
        bq8, bk, bo, b2 = par[:, 32:40], par[:, 40:48], par[:, 48:56], \
            par[:, 56:64]
        b1 = par[:, 64:96]
        bv = param_p.tile([1, D], BF16, tag="bv", name="bv")
        nc.scalar.dma_start(bv[:], bv_d[:])
        # bv broadcast across partitions (added to V tiles with a DVE add)
        bvb = param_p.tile([128, D], BF16, tag="bvb", name="bvb")
        nc.gpsimd.partition_broadcast(bvb[:], bv[:])

        res = [res_p.tile([128, IB], F32, tag=f"res{e}", name=f"res{e}")
               for e in range(8)]

        # ------- scope: K/V/Q + attention + out-proj (phases A-D1) ---------
        with tc.tile_pool(name="kT", bufs=1) as kT_p, \
             tc.tile_pool(name="vcat", bufs=1) as vcat_p, \
             tc.tile_pool(name="qT", bufs=1) as qT_p:
            kT = [kT_p.tile([128, T], BF16, tag=f"kT{d}", name=f"kT{d}")
                  for d in range(8)]
            vcat = [vcat_p.tile([128, H * (HD + 1)], BF16, tag=f"vc{t}",
                                name=f"vc{t}") for t in range(16)]
            qT = [qT_p.tile([128, IB], BF16, tag=f"qT{d}", name=f"qT{d}")
                  for d in range(8)]

            # ===== Phases A+B interleaved: LN1(n) -> [Q] -> K(n) -> V(n) ===
            # LN stats run one block ahead so the mu/rstd chain overlaps the
            # previous block's K/V matmuls.
            with tc.tile_pool(name="wqkv", bufs=1) as wqkv_p, \
                 tc.tile_pool(name="xc", bufs=2) as xc_p, \
                 tc.tile_pool(name="hT", bufs=2) as hT_p, \
                 tc.tile_pool(name="sq", bufs=2) as sq_p, \
                 tc.tile_pool(name="lnt", bufs=1) as lnt_p, \
                 tc.tile_pool(name="lnb", bufs=2) as lnb_p, \
                 tc.tile_pool(name="lnps", bufs=2,
                              space=bass.MemorySpace.PSUM) as lnps_p, \
                 tc.tile_pool(name="pps", bufs=4,
                              space=bass.MemorySpace.PSUM) as pps:
                # ones columns of vcat (memset before V writes the rest)
                for tt in range(16):
                    nc.vector.memset(
                        vcat[tt][:].rearrange(
                            "p (h x) -> p h x", x=HD + 1)[:, :, HD:HD + 1],
                        1.0)

                wqkv = {}

                def load_w(nm, dram):
                    t = wqkv_p.tile([128, 8 * D], BF16, tag=nm, name=nm)
                    nc.sync.dma_start(t[:], dram[:])
                    wqkv[nm] = t

                def ln_stats(n):
                    nb = slice(n * 512, (n + 1) * 512)
                    xcs, stats = [], {}
                    ps_mu = lnps_p.tile([1, 512], F32, tag="psmu", name="psmu")
                    ps_sq = lnps_p.tile([1, 512], F32, tag="pssq", name="pssq")
                    for e in range(8):
                        xc = xc_p.tile([128, 512], BF16, tag=f"xc{e}",
                                       name="xc")
                        nc.sync.dma_start(xc[:],
                                          xbT_d[e * 128:(e + 1) * 128, nb])
                        xcs.append(xc)
                        nc.tensor.matmul(ps_mu[:], ones_c[:], xc[:],
                                         start=(e == 0), stop=(e == 7))
                        x2 = sq_p.tile([128, 512], BF16, tag="x2", name="x2")
                        nc.vector.tensor_mul(x2[:], xc[:], xc[:])
                        nc.tensor.matmul(ps_sq[:], ones_c[:], x2[:],
                                         start=(e == 0), stop=(e == 7))
                    stats["mu"], stats["sq"], stats["xcs"] = ps_mu, ps_sq, xcs
                    return stats

                def ln_normalize(st):
                    mu_n = lnt_p.tile([1, 512], F32, tag="mu_n", name="mu_n")
                    nc.scalar.activation(mu_n[:], st["mu"][:], AF.Identity,
                                         scale=1.0 / D)
                    t1 = lnt_p.tile([1, 512], F32, tag="t1", name="t1")
                    nc.vector.tensor_mul(t1[:], mu_n[:], mu_n[:])
                    t2 = lnt_p.tile([1, 512], F32, tag="t2", name="t2")
                    nc.vector.scalar_tensor_tensor(
                        t2[:], st["sq"][:], 1.0 / D, t1[:],
                        op0=OP.mult, op1=OP.subtract)
                    nc.scalar.activation(t1[:], t2[:], AF.Sqrt, bias=eps_t[:])
                    nc.vector.reciprocal(t2[:], t1[:])
                    mu_b = lnb_p.tile([128, 512], F32, tag="mu_b", name="mu_b")
                    nc.gpsimd.partition_broadcast(mu_b[:], mu_n[:])
                    rstd_b = lnb_p.tile([128, 512], F32, tag="rstd_b",
                                        name="rstd_b")
                    nc.gpsimd.partition_broadcast(rstd_b[:], t2[:])
                    hTn = []
                    for e in range(8):
                        t = sq_p.tile([128, 512], F32, tag="lnap", name="lnap")
                        nc.vector.tensor_sub(t[:], st["xcs"][e][:], mu_b[:])
                        nc.vector.tensor_mul(t[:], t[:], rstd_b[:])
                        h = hT_p.tile([128, 512], BF16, tag=f"hT{e}",
                                      name=f"hT{e}")
                        nc.scalar.activation(h[:], t[:], AF.Identity,
                                             scale=g1[:, e:e + 1],
                                             bias=bg1[:, e:e + 1])
                        hTn.append(h)
                    return hTn

                st = ln_stats(0)
                load_w("wq", Wq_d)
                st_next = ln_stats(1)
                load_w("wk", Wk_d)
                load_w("wv", Wv_d)
                for n in range(4):
                    nb = slice(n * 512, (n + 1) * 512)
                    hTn = ln_normalize(st)
                    st = st_next
                    if n + 2 < 4:
                        st_next = ln_stats(n + 2)

                    # ---- Q projection (query block == token cols 0:512) ----
                    if n == 0:
                        wq = wqkv["wq"]
                        for dt in range(8):
                            ps = pps.tile([128, 512], F32, tag="ps", name="psq")
                            for e in range(8):
                                nc.tensor.matmul(
                                    ps[:],
                                    wq[:, e * D + dt * 128:e * D + (dt + 1) * 128],
                                    hTn[e][:], start=(e == 0), stop=(e == 7))
                            nc.scalar.activation(qT[dt][:], ps[:], AF.Identity,
                                                 scale=SCALE,
                                                 bias=bq8[:, dt:dt + 1])

                    # ---- K projection for this token block ----
                    wk = wqkv["wk"]
                    for dt in range(8):
                        ps = pps.tile([128, 512], F32, tag="ps", name="psk")
                        for e in range(8):
                            nc.tensor.matmul(
                                ps[:],
                                wk[:, e * D + dt * 128:e * D + (dt + 1) * 128],
                                hTn[e][:], start=(e == 0), stop=(e == 7))
                        nc.vector.tensor_scalar_add(kT[dt][:, nb], ps[:],
                                                    bk[:, dt:dt + 1])

                    # ---- V for this token block (both feature halves) ----
                    wv = wqkv["wv"]
                    for tt in range(n * 4, n * 4 + 4):
                        tb = slice((tt - n * 4) * 128, (tt - n * 4 + 1) * 128)
                        for n2 in range(2):
                            n2b = slice(n2 * 512, (n2 + 1) * 512)
                            ps = pps.tile([128, 512], F32, tag="ps", name="psv")
                            for e in range(8):
                                nc.tensor.matmul(
                                    ps[:], hTn[e][:, tb],
                                    wv[:, e * D + n2 * 512:
                                       e * D + (n2 + 1) * 512],
                                    start=(e == 0), stop=(e == 7))
                            dst = vcat[tt][:, n2 * 8 * (HD + 1):
                                           (n2 + 1) * 8 * (HD + 1)]
                            dst = dst.rearrange("p (h x) -> p h x",
                                                x=HD + 1)[:, :, 0:HD]
                            src = ps[:].rearrange("p (h d) -> p h d", d=HD)
                            bvs = bvb[:, n2b].rearrange("p (h d) -> p h d",
                                                        d=HD)
                            nc.vector.tensor_add(dst, src, bvs)

            # ===== Phase C: attention (heads paired per kT tile) ===========
            # scores for both heads of a pair land in one [128,1024] PSUM
            # tile (2 banks); exp is one fused ACT op; the exp(bias) multiply
            # is split across DVE (head 0) and Pool (head 1).
            with tc.tile_pool(name="wo", bufs=1) as wo_p, \
                 tc.tile_pool(name="oT", bufs=1) as oT_p, \
                 tc.tile_pool(name="ebias", bufs=3) as eb_p, \
                 tc.tile_pool(name="uexp", bufs=3) as ue_p, \
                 tc.tile_pool(name="umul", bufs=3) as u_p, \
                 tc.tile_pool(name="nrm", bufs=2) as nrm_p, \
                 tc.tile_pool(name="pss", bufs=2,
                              space=bass.MemorySpace.PSUM) as pss, \
                 tc.tile_pool(name="pso", bufs=4,
                              space=bass.MemorySpace.PSUM) as pso:
                oT = [oT_p.tile([128, IB], BF16, tag=f"oT{d}", name=f"oT{d}")
                      for d in range(8)]
                wo = wo_p.tile([128, 8 * D], BF16, tag="wo", name="wo")

                ebs = {}

                def load_eb(h):
                    t = eb_p.tile([128, 16 * IB], F8, tag="eb", name=f"eb{h}")
                    nc.sync.dma_start(t[:], ebT_d[h])
                    ebs[h] = t

                load_eb(0)
                load_eb(1)
                load_eb(2)
                for p in range(8):
                    eb0, eb1 = ebs.pop(2 * p), ebs.pop(2 * p + 1)
                    ps_o = [pso.tile([HD + 1, 512], F32, tag="ps_o",
                                     name="ps_o") for _ in range(2)]
                    us = [None] * 16

                    def attn_av(j):
                        # attn@V for block j (emitted one step behind the
                        # scores/exp chain so the PE FIFO never blocks on
                        # the current block's exp)
                        for s in range(2):
                            h = 2 * p + s
                            nc.tensor.matmul(
                                ps_o[s][:],
                                vcat[j][:, h * (HD + 1):(h + 1) * (HD + 1)],
                                us[j][:, s * 512:(s + 1) * 512],
                                start=(j == 0), stop=(j == 15))

                    for j in range(16):
                        jb = slice(j * 128, (j + 1) * 128)
                        ps_s = pss.tile([128, 1024], F32, tag="ps_s",
                                        name="ps_s")
                        for s in range(2):
                            po = s * 64
                            nc.tensor.matmul(ps_s[:, s * 512:(s + 1) * 512],
                                             kT[p][po:po + 64, jb],
                                             qT[p][po:po + 64, :],
                                             start=True, stop=True)
                        ue = ue_p.tile([128, 1024], BF16, tag="ue", name="ue")
                        nc.scalar.activation(ue[:], ps_s[:], AF.Exp)
                        u = u_p.tile([128, 1024], BF16, tag="u", name="u")
                        nc.vector.tensor_mul(u[:, 0:512], ue[:, 0:512],
                                             eb0[:, j * IB:(j + 1) * IB])
                        nc.gpsimd.tensor_mul(u[:, 512:1024], ue[:, 512:1024],
                                             eb1[:, j * IB:(j + 1) * IB])
                        us[j] = u
                        if j > 0:
                            attn_av(j - 1)
                    attn_av(15)
                    if 2 * p + 3 < H:
                        load_eb(2 * p + 3)
                    for s in range(2):
                        po = s * 64
                        recip = nrm_p.tile([1, 512], F32, tag="recip",
                                           name="recip")
                        nc.vector.reciprocal(recip[:], ps_o[s][64:65, :])
                        rb = nrm_p.tile([64, 512], F32, tag="rb", name="rb")
                        nc.gpsimd.partition_broadcast(rb[:], recip[:])
                        nc.vector.tensor_mul(oT[p][po:po + 64, :],
                                             ps_o[s][0:64, :], rb[:])
                    if 2 * p + 4 < H:
                        load_eb(2 * p + 4)
                    if p == 0:
                        # x residual rows for the query block (for out-proj)
                        for e in range(8):
                            nc.sync.dma_start(res[e][:],
                                              xT_d[e * 128:(e + 1) * 128, 0:IB])
                    elif p == 1:
                        nc.sync.dma_start(wo[:], Wo_d[:])

                # ---- out-projection (reuses pss PSUM pool; inside C scope
                #      so wo/oT stay live) ----
                for et in range(8):
                    ps = pss.tile([128, 1024], F32, tag="ps_s", name="psx1")
                    for dt in range(8):
                        nc.tensor.matmul(
                            ps[:, 0:512],
                            wo[:, dt * D + et * 128:dt * D + (et + 1) * 128],
                            oT[dt][:], start=(dt == 0), stop=(dt == 7))
                    # res[et] <- x + attn_out (+bo), in place
                    nc.vector.scalar_tensor_tensor(res[et][:], ps[:, 0:512],
                                                   bo[:, et:et + 1], res[et][:],
                                                   op0=OP.add, op1=OP.add)

        # ---------------- scope: LN2 + FFN (phases D2/E) -------------------
        with tc.tile_pool(name="h2", bufs=1) as h2_p, \
             tc.tile_pool(name="w1s", bufs=2) as w1s_p, \
             tc.tile_pool(name="w2s", bufs=2) as w2s_p, \
             tc.tile_pool(name="sz", bufs=1) as sz_p, \
             tc.tile_pool(name="out", bufs=2) as out_p, \
             tc.tile_pool(name="sq2", bufs=2) as sq2_p, \
             tc.tile_pool(name="lnt2", bufs=1) as lnt2_p, \
             tc.tile_pool(name="lnb2", bufs=1) as lnb2_p, \
             tc.tile_pool(name="dps1", bufs=2,
                          space=bass.MemorySpace.PSUM) as dps1, \
             tc.tile_pool(name="eps", bufs=4,
                          space=bass.MemorySpace.PSUM) as eps_p:
            w1cs = {}

            def load_w1(c):
                t = w1s_p.tile([128, 8192], BF16, tag="w1c", name=f"w1c{c}")
                nc.sync.dma_start(t[:], W1_d[:, c * 8192:(c + 1) * 8192])
                w1cs[c] = t

            w2cs = {}

            def load_w2(c):
                t = w2s_p.tile([128, 8192], BF16, tag="w2c", name=f"w2c{c}")
                nc.sync.dma_start(t[:], W2_d[:, c * 8192:(c + 1) * 8192])
                w2cs[c] = t

            load_w1(0)
            load_w1(1)

            h2 = [h2_p.tile([128, IB], BF16, tag=f"h2{e}", name=f"h2{e}")
                  for e in range(8)]

            # LN2 (single 512-col block)
            ps_mu = dps1.tile([1, 512], F32, tag="psmu2", name="psmu2")
            for e in range(8):
                xb = sq2_p.tile([128, 512], BF16, tag="xb2", name="xb2")
                nc.gpsimd.tensor_copy(xb[:], res[e][:])
                nc.tensor.matmul(ps_mu[:], ones_c[:], xb[:],
                                 start=(e == 0), stop=(e == 7))
            ps_sq = dps1.tile([1, 512], F32, tag="pssq2", name="pssq2")
            for e in range(8):
                x2 = sq2_p.tile([128, 512], BF16, tag="x22", name="x22")
                nc.gpsimd.tensor_mul(x2[:], res[e][:], res[e][:])
                nc.tensor.matmul(ps_sq[:], ones_c[:], x2[:],
                                 start=(e == 0), stop=(e == 7))
            mu_n = lnt2_p.tile([1, 512], F32, tag="mu_n2", name="mu_n2")
            nc.scalar.activation(mu_n[:], ps_mu[:], AF.Identity, scale=1.0 / D)
            t1 = lnt2_p.tile([1, 512], F32, tag="t1_2", name="t1_2")
            nc.vector.tensor_mul(t1[:], mu_n[:], mu_n[:])
            t2 = lnt2_p.tile([1, 512], F32, tag="t2_2", name="t2_2")
            nc.vector.scalar_tensor_tensor(t2[:], ps_sq[:], 1.0 / D,
                                           t1[:], op0=OP.mult,
                                           op1=OP.subtract)
            nc.scalar.activation(t1[:], t2[:], AF.Sqrt, bias=eps_t[:])
            nc.vector.reciprocal(t2[:], t1[:])
            mu_b = lnb2_p.tile([128, 512], F32, tag="mu_b2", name="mu_b2")
            nc.gpsimd.partition_broadcast(mu_b[:], mu_n[:])
            rstd_b = lnb2_p.tile([128, 512], F32, tag="rstd_b2",
                                 name="rstd_b2")
            nc.gpsimd.partition_broadcast(rstd_b[:], t2[:])
            for e in range(8):
                t = sq2_p.tile([128, IB], F32, tag="lnap2", name="lnap2")
                nc.vector.tensor_sub(t[:], res[e][:], mu_b[:])
                nc.vector.tensor_mul(t[:], t[:], rstd_b[:])
                nc.scalar.activation(h2[e][:], t[:], AF.Identity,
                                     scale=g2[:, e:e + 1],
                                     bias=bg2[:, e:e + 1])

            # ===== Phase E: FFN ============================================
            sz = [sz_p.tile([128, IB], BF16, tag=f"sz{f}", name=f"sz{f}")
                  for f in range(32)]
            for ft in range(32):
                c = ft // 8
                if ft == 0:
                    load_w2(0)
                    load_w2(1)
                w1t = w1cs[c]
                ps = eps_p.tile([128, 512], F32, tag="pse", name="psz")
                for e in range(8):
                    nc.tensor.matmul(
                        ps[:],
                        w1t[:, (ft % 8) * 1024 + e * 128:
                            (ft % 8) * 1024 + (e + 1) * 128],
                        h2[e][:], start=(e == 0), stop=(e == 7))
                if SILU_ON_ACT:
                    nc.scalar.activation(sz[ft][:], ps[:], AF.Silu,
                                         bias=b1[:, ft:ft + 1])
                else:
                    sg = sq2_p.tile([128, IB], BF16, tag="sg", name="sg")
                    nc.scalar.activation(sg[:], ps[:], AF.Sigmoid,
                                         bias=b1[:, ft:ft + 1])
                    nc.vector.scalar_tensor_tensor(sz[ft][:], ps[:],
                                                   b1[:, ft:ft + 1], sg[:],
                                                   op0=OP.add, op1=OP.mult)
                if ft % 8 == 7 and c + 2 < 4:
                    load_w1(c + 2)
            for et in range(8):
                c = et // 2
                w2t = w2cs[c]
                ps = eps_p.tile([128, 512], F32, tag="pse", name="psy")
                for ft in range(32):
                    nc.tensor.matmul(
                        ps[:],
                        w2t[:, (et % 2) * 4096 + ft * 128:
                            (et % 2) * 4096 + (ft + 1) * 128],
                        sz[ft][:], start=(ft == 0), stop=(ft == 31))
                ot = out_p.tile([128, IB], F32, tag="outt", name="outt")
                nc.vector.scalar_tensor_tensor(ot[:], ps[:],
                                               b2[:, et:et + 1],
                                               res[et][:], op0=OP.add,
                                               op1=OP.add)
                nc.sync.dma_start(outT_d[et * 128:(et + 1) * 128, :], ot[:])
                if et % 2 == 1 and c + 2 < 4:
                    load_w2(c + 2)

    nc.compile()
    return nc


def _prep_inputs(inputs):
    """Host-side layout prep -> list of 8 per-core input maps."""
    x = np.asarray(inputs["x"], dtype=np.float32)
    ab = np.asarray(inputs["attn_bias"], dtype=np.float32)

    def pack(v, ntiles):
        return np.ascontiguousarray(
            np.asarray(v, np.float32).reshape(ntiles, 128).T)

    def packw(w, ntiles):  # [ntiles*128, N] -> [128, ntiles*N]
        w = np.asarray(w).astype(BF16_NP)
        n = w.shape[1]
        return np.ascontiguousarray(
            w.reshape(ntiles, 128, n).transpose(1, 0, 2).reshape(
                128, ntiles * n))

    # W1 is chunk-loaded in ft-major quarters: [p, ft*1024 + e*128 + j]
    w1 = np.asarray(inputs["W1"]).astype(BF16_NP)
    w1p = np.ascontiguousarray(
        w1.reshape(8, 128, 32, 128).transpose(1, 2, 0, 3).reshape(128, 32768))
    # W2 is chunk-loaded in et-major quarters: [p, et*4096 + ft*128 + j]
    w2 = np.asarray(inputs["W2"]).astype(BF16_NP)
    w2p = np.ascontiguousarray(
        w2.reshape(32, 128, 8, 128).transpose(1, 2, 0, 3).reshape(128, 32768))

    par = np.concatenate([
        pack(inputs["ln1_g"], 8), pack(inputs["ln1_b"], 8),
        pack(inputs["ln2_g"], 8), pack(inputs["ln2_b"], 8),
        pack(np.asarray(inputs["bq"], np.float32) * SCALE, 8),
        pack(inputs["bk"], 8), pack(inputs["bo"], 8), pack(inputs["b2"], 8),
        pack(inputs["b1"], 32)], axis=1)

    shared = {
        "Wq": packw(inputs["Wq"], 8),
        "Wk": packw(inputs["Wk"], 8),
        "Wv": packw(inputs["Wv"], 8),
        "Wo": packw(inputs["Wo"], 8),
        "W1": w1p,
        "W2": w2p,
        "par": np.ascontiguousarray(par),
        "bv": np.ascontiguousarray(
            np.asarray(inputs["bv"], np.float32).astype(BF16_NP).reshape(1, D)),
    }
    xT = [np.ascontiguousarray(x[b].T) for b in range(B)]  # [D, T] f32
    in_maps = []
    for c in range(N_CORES):
        b, i0 = c // 4, (c % 4) * IB
        # token axis rotated by -i0 (queries land at cols 0:IB); the j axis
        # of the bias is rotated identically to match k/v token order.
        xTc = np.ascontiguousarray(np.roll(xT[b], -i0, axis=1))
        # exp(bias)/16 as fp8, laid out [pair, 128, (s*16 + jblk)*512 + i]
        ebc = np.exp(ab[b, :, i0:i0 + IB, :]) * EB_SCALE      # [H, 512i, 2048j]
        ebc = np.roll(ebc, -i0, axis=2).transpose(0, 2, 1)    # [H, 2048j, 512i]
        ebc = ebc.reshape(H, 16, 128, IB).transpose(0, 2, 1, 3).reshape(
            H, 128, 16 * IB)                                  # [H, 128, 8192]
        m = {"xT": xTc, "xb": np.ascontiguousarray(xTc.astype(BF16_NP)),
             "ebT": np.ascontiguousarray(ebc.astype(F8_NP))}
        m.update(shared)
        in_maps.append(m)
    return in_maps


def kernel(**inputs):
    if "nc" not in _cache:
        _cache["nc"] = build_program()
    nc = _cache["nc"]
    in_maps = _prep_inputs(inputs)
    r = run_bass_kernel_spmd(nc, in_maps, list(range(N_CORES)))
    out = np.empty((B, T, D), dtype=np.float32)
    for c in range(N_CORES):
        b, i0 = c // 4, (c % 4) * IB
        out[b, i0:i0 + IB, :] = np.asarray(r.results[c]["outT"], np.float32).T
    return out
